# revision 16
# baseline (speedup 1.0000x reference)
"""Trainium2 Bass kernel for dual channel-attention block (nn_Attention_85985245266248).

Strategy:
  - Shard spatially: 256 rows -> 8 cores x 32 rows, each core's input shard
    carries a 1-row halo (zero at global edges) and 1-col zero padding.
  - Axon-tunnel traffic is the wall-clock bottleneck (~38MB/s), so all
    transfers are quantized: inputs int8 with per-(batch,channel) scales
    (dequantized on device), outputs int8 with per-(core,batch,channel)
    scales (quantized on device, round-to-nearest), weights bf16 sharded
    1/8th per core and AllGather'd on device.
  - conv1x1 + depthwise3x3 folded into a full 3x3 conv (rank-1 weights),
    executed as 9 PSUM-accumulated matmuls per tile on the PE.
  - Pass A computes q,k in [px, ch] layout (input stationary, weights moving)
    so the c-x-c Gram matrices q@k^T and the L2 norms come straight off the
    PE with pixel-contraction; partial Grams are AllReduce'd across cores.
  - Pass B computes v in [ch, px] layout (weights stationary).
  - Softmax + norm scaling on DVE/ACT (tiny 96x96 tensors).
  - Output projection po/concat folded on host into P_c/P_t; final output is
    two accumulated matmuls per pixel chunk: out = M_cT^T @ v_t + M_tT^T @ v_c + b.
All heavy matmuls run in bf16 (fp32 accumulate in PSUM).
"""
import os
import sys
import numpy as np

sys.path.insert(0, "/opt/trn_rl_repo")

B = 2
D = 96
H = 256
W = 256
HEADS = 3
NC = 8
RPC = H // NC          # rows per core = 32
HR = RPC + 2           # halo rows = 34
PW = W + 2             # padded width = 258
PXT = 128              # pass-A pixel tile (half row)
NT_A = RPC * W // PXT  # pass-A tiles per batch per tensor = 64
CHK = 512              # pass-B / final chunk = 2 rows
NCHK = RPC * W // CHK  # 16

# flat bf16 weight-gather layout: (name, elems)
WPACK = [
    ("wqk_hi", D * 9 * 2 * D),   # 165888
    ("wqk_lo", D * 9 * 2 * D),   # 165888
    ("wv_hi", D * 9 * D),        # 82944
    ("wv_lo", D * 9 * D),        # 82944
    ("pct", D * D),              # 9216
    ("ptt", D * D),              # 9216
    ("ident", D * D),            # 9216
]
WTOT = sum(n for _, n in WPACK)  # 525312
WSH = WTOT // NC                 # 65664 per core

# single-blob input layout (bytes). One sharded array per dispatch kills
# the ~70ms-per-array axon fixed cost.
XSZ1 = D * HR * W              # one (tensor, batch) block, no col pad
WOFF = 4 * XSZ1                # bf16 weight shard bytes
SOFF = WOFF + WSH * 2          # f32 smalls: sc_hi(2), sc_lo(2), tempv, biasv
NSM = 2 * B + 2
NB = SOFF + NSM * D * 4        # total blob bytes per core
OCOL = RPC * W + 4             # int8 payload + bitcast f32 scale per row

_CACHE = {}


def _fold3x3(w1, dw):
    """w1:[O,C], dw:[O,1,3,3] -> [9, C, O] rhs-layout folded weights."""
    O, C = w1.shape
    out = np.zeros((9, C, O), np.float32)
    for t in range(9):
        dy, dx = t // 3, t % 3
        out[t] = (dw[:, 0, dy, dx][:, None] * w1).T
    return out


def _bf16(a):
    import ml_dtypes
    return np.asarray(a, np.float32).astype(ml_dtypes.bfloat16)


def _build(nc_mod):
    """Build the Bass program (uses modules passed in)."""
    bass, bacc, tile, mybir = nc_mod
    f32 = mybir.dt.float32
    bf16 = mybir.dt.bfloat16
    i8 = mybir.dt.int8

    nc = bacc.Bacc("TRN2", target_bir_lowering=False, debug=False, num_devices=NC)

    # I/O: one flat int8 blob in (x shards + bf16 weight shard + f32 smalls,
    # all bitcast), one int8 blob out (payload + bitcast f32 scale per row).
    xin = nc.dram_tensor("xin", [1, NB], i8, kind="ExternalInput")
    oq8 = nc.dram_tensor("oq8", [B, D, OCOL], i8, kind="ExternalOutput")

    NG = 6  # grams per batch: G1, G2, Sqc, Skc, Sqt, Skt

    with tile.TileContext(nc) as tc:
        with (
            tc.tile_pool(name="consts", bufs=1) as cpool,
            tc.tile_pool(name="xq", bufs=1) as xqpool,
            tc.tile_pool(name="xres", bufs=1) as xpool,
            tc.tile_pool(name="vres", bufs=1) as vpool,
            tc.tile_pool(name="qk", bufs=4) as qkpool,
            tc.tile_pool(name="work_ps", bufs=3, space="PSUM") as wps,
            tc.tile_pool(name="gram_ps", bufs=1, space="PSUM") as gps,
            tc.tile_pool(name="small", bufs=1) as spool,
            tc.tile_pool(name="obuf", bufs=1) as opool,
            tc.tile_pool(name="dram", bufs=1, space="DRAM") as dpool,
        ):
            # ---- weight AllGather: 1/8th slice per core -> full flat ----
            wsh_sb = cpool.tile([D, WSH // D], bf16, tag="wsh")
            nc.sync.dma_start(out=wsh_sb[:],
                              in_=xin[0, WOFF:WOFF + WSH * 2].bitcast(bf16))
            wag_in = dpool.tile([1, WSH], bf16, tag="wagin")
            wag_out = dpool.tile([1, WTOT], bf16, tag="wagout")
            nc.gpsimd.dma_start(out=wag_in[:], in_=wsh_sb[:])
            nc.gpsimd.collective_compute(
                "AllGather",
                mybir.AluOpType.bypass,
                replica_groups=[list(range(NC))],
                ins=[wag_in.opt()],
                outs=[wag_out.opt()],
            )

            # ---- unpack gathered weights into const tiles ----
            wqk_hi_sb = cpool.tile([D, 9, 2 * D], bf16, tag="wqkh")
            wqk_lo_sb = cpool.tile([D, 9, 2 * D], bf16, tag="wqkl")
            wv_hi_sb = cpool.tile([D, 9, D], bf16, tag="wvh")
            wv_lo_sb = cpool.tile([D, 9, D], bf16, tag="wvl")
            pct_sb = cpool.tile([D, D], bf16, tag="pct")
            ptt_sb = cpool.tile([D, D], bf16, tag="ptt")
            identb_sb = cpool.tile([D, D], bf16, tag="identb")
            wtiles = {"wqk_hi": wqk_hi_sb, "wqk_lo": wqk_lo_sb,
                      "wv_hi": wv_hi_sb, "wv_lo": wv_lo_sb,
                      "pct": pct_sb, "ptt": ptt_sb, "ident": identb_sb}
            off = 0
            for nm, n in WPACK:
                nc.gpsimd.dma_start(out=wtiles[nm][:], in_=wag_out[0, off:off + n])
                off += n
            ident_sb = cpool.tile([D, D], f32, tag="ident")
            nc.vector.tensor_copy(ident_sb[:], identb_sb[:])

            def small_slice(i):
                o = SOFF + i * D * 4
                return xin[0, o:o + D * 4].bitcast(f32)

            scq_sb = {}
            for i, (s, b) in enumerate((("hi", 0), ("hi", 1),
                                        ("lo", 0), ("lo", 1))):
                t = cpool.tile([D, 1], f32, tag=f"sc{s}{b}")
                nc.sync.dma_start(out=t[:], in_=small_slice(i))
                scq_sb[(s, b)] = t
            tempv_sb = cpool.tile([D, 1], f32, tag="tempv")
            biasv_sb = cpool.tile([D, 1], f32, tag="biasv")
            nc.sync.dma_start(out=tempv_sb[:], in_=small_slice(4))
            nc.sync.dma_start(out=biasv_sb[:], in_=small_slice(5))

            # gram accumulation targets and per-batch v stores
            gram_cat = spool.tile([D, B * NG * D], f32, tag="gramcat")
            v_sb = {}   # (b, 'hi'/'lo') -> [D, RPC*W] bf16
            for b in range(B):
                for s in ("hi", "lo"):
                    v_sb[(b, s)] = vpool.tile([D, RPC * W], bf16,
                                              tag=f"v{b}{s}", name=f"v{b}{s}")

            xt = {}
            for b in range(B):
                # ---- load + dequantize this batch's input shards ----
                for si, s in enumerate(("hi", "lo")):
                    xoff = (si * B + b) * XSZ1
                    xq = xqpool.tile([D, HR, PW], i8, tag="xq")
                    nc.vector.memset(xq[:], 0.0)
                    nc.sync.dma_start(out=xq[:, :, 1:W + 1],
                                      in_=xin[0, xoff:xoff + XSZ1])
                    xd = xpool.tile([D, HR, PW], bf16, tag=f"x{s}")
                    nc.scalar.activation(xd[:], xq[:],
                                         mybir.ActivationFunctionType.Identity,
                                         bias=0.0, scale=scq_sb[(s, b)][:])
                    xt[(b, s)] = xd
                    del xq, xd

                # ---- pass A: q,k in [px, ch] + Gram/norm accumulation ----
                # paired layout sbp[:, g, :]: g=0 -> [q_c | k_t], g=1 -> [k_c | q_t]
                gA = gps.tile([D, 2 * D], f32, tag="gA", name=f"gA{b}")  # [Sqc | G1]
                gB = gps.tile([D, 2 * D], f32, tag="gB", name=f"gB{b}")  # [G2 | Sqt]
                gC = gps.tile([D, D], f32, tag="gC", name=f"gC{b}")      # Skt
                gD = gps.tile([D, D], f32, tag="gD", name=f"gD{b}")      # Skc

                def grams(sbp, first, last):
                    nc.tensor.matmul(gA[:], sbp[:, 0, 0:D], sbp[:, 0, :],
                                     start=first, stop=last)
                    nc.tensor.matmul(gB[:], sbp[:, 1, D:2 * D], sbp[:, 1, :],
                                     start=first, stop=last)
                    nc.tensor.matmul(gC[:], sbp[:, 0, D:2 * D], sbp[:, 0, D:2 * D],
                                     start=first, stop=last)
                    nc.tensor.matmul(gD[:], sbp[:, 1, 0:D], sbp[:, 1, 0:D],
                                     start=first, stop=last)

                prev = None
                for it in range(NT_A):
                    r = (it * PXT) // W          # output row 0..31
                    j = (it * PXT) % W           # 0 or 128
                    sbp = qkpool.tile([PXT, 2, 2 * D], bf16, tag="qksb")
                    for gi, (s, wsb) in enumerate((("hi", wqk_hi_sb),
                                                   ("lo", wqk_lo_sb))):
                        ps = wps.tile([PXT, 2 * D], f32, tag="apsum")
                        xs = xt[(b, s)]
                        for t in range(9):
                            dy, dx = t // 3, t % 3
                            lhsT = xs[:, r + dy, j + dx:j + dx + PXT]
                            nc.tensor.matmul(ps[:], lhsT, wsb[:, t, :],
                                             start=(t == 0), stop=(t == 8))
                        # hi [q_c|k_c] -> cols {0:96, 192:288}; lo [k_t|q_t] -> {96:192, 288:384}
                        nc.vector.tensor_copy(sbp[:, :, gi * D:(gi + 1) * D], ps[:])
                    if prev is not None:
                        grams(prev, prev_first, False)
                    prev_first = prev is None
                    prev = sbp
                grams(prev, False, True)

                for k, src in (("G1", gA[:, D:2 * D]), ("G2", gB[:, 0:D]),
                               ("Sqc", gA[:, 0:D]), ("Skc", gD[:]),
                               ("Sqt", gB[:, D:2 * D]), ("Skt", gC[:])):
                    gi = ("G1", "G2", "Sqc", "Skc", "Sqt", "Skt").index(k)
                    off = (b * NG + gi) * D
                    nc.vector.tensor_copy(gram_cat[:, off:off + D], src)

                # ---- pass B: v in [ch, px] ----
                for s, wsb in (("hi", wv_hi_sb), ("lo", wv_lo_sb)):
                    xs = xt[(b, s)]
                    for ck in range(NCHK):
                        r = ck * 2
                        ps = wps.tile([D, CHK], f32, tag="apsum")
                        for t in range(9):
                            dy, dx = t // 3, t % 3
                            rhs = xs[:, r + dy:r + dy + 2, dx:dx + W]
                            nc.tensor.matmul(ps[:], wsb[:, t, :], rhs,
                                             start=(t == 0), stop=(t == 8))
                        nc.vector.tensor_copy(
                            v_sb[(b, s)][:, ck * CHK:(ck + 1) * CHK], ps[:])

            # ---- AllReduce partial grams across the 8 cores ----
            ar_in = dpool.tile([D, B * NG * D], f32, tag="arin")
            ar_out = dpool.tile([D, B * NG * D], f32, tag="arout")
            nc.gpsimd.dma_start(out=ar_in[:], in_=gram_cat[:])
            nc.gpsimd.collective_compute(
                "AllReduce",
                mybir.AluOpType.add,
                replica_groups=[list(range(NC))],
                ins=[ar_in.opt()],
                outs=[ar_out.opt()],
            )
            gram_red = spool.tile([D, B * NG * D], f32, tag="gramred")
            nc.gpsimd.dma_start(out=gram_red[:], in_=ar_out[:])

            # ---- post-AR small compute per batch ----
            mt = {}  # (b, 'c'/'t') -> M^T tile [D, D] bf16
            for b in range(B):
                def gslice(gi):
                    off = (b * NG + gi) * D
                    return gram_red[:, off:off + D]
                G1, G2, Sqc, Skc, Sqt, Skt = [gslice(i) for i in range(NG)]

                rcol = {}
                for nm, S in (("qc", Sqc), ("kc", Skc), ("qt", Sqt), ("kt", Skt)):
                    tmp = spool.tile([D, D], f32, tag="dtmp")
                    nc.vector.tensor_tensor(out=tmp[:], in0=S, in1=ident_sb[:],
                                            op=mybir.AluOpType.mult)
                    dg = spool.tile([D, 1], f32, tag=f"d{nm}{b}")
                    nc.vector.tensor_reduce(out=dg[:], in_=tmp[:],
                                            axis=mybir.AxisListType.X,
                                            op=mybir.AluOpType.add)
                    sq = spool.tile([D, 1], f32, tag=f"sq{nm}{b}")
                    nc.scalar.sqrt(sq[:], dg[:])
                    rc = spool.tile([D, 1], f32, tag=f"rc{nm}{b}")
                    nc.vector.reciprocal(rc[:], sq[:])
                    rcol[nm] = rc
                # fold temperature into rq
                for nm in ("qc", "qt"):
                    nc.vector.tensor_tensor(out=rcol[nm][:], in0=rcol[nm][:],
                                            in1=tempv_sb[:],
                                            op=mybir.AluOpType.mult)

                # row-vector 1/||k|| via partition reduce of (S*I)
                rrow = {}
                for nm, S in (("kt", Skt), ("kc", Skc)):
                    tmp = spool.tile([D, D], f32, tag="dtmp")
                    nc.vector.tensor_tensor(out=tmp[:], in0=S, in1=ident_sb[:],
                                            op=mybir.AluOpType.mult)
                    drow = spool.tile([1, D], f32, tag=f"dr{nm}{b}")
                    nc.gpsimd.tensor_reduce(out=drow[:], in_=tmp[:],
                                            axis=mybir.AxisListType.C,
                                            op=mybir.AluOpType.add)
                    sqr = spool.tile([1, D], f32, tag=f"sqr{nm}{b}")
                    nc.scalar.sqrt(sqr[:], drow[:])
                    rr = spool.tile([1, D], f32, tag=f"rr{nm}{b}")
                    nc.vector.reciprocal(rr[:], sqr[:])
                    rb = spool.tile([D, D], f32, tag=f"rb{nm}{b}")
                    nc.gpsimd.partition_broadcast(rb[:], rr[:])
                    rrow[nm] = rb

                for attn_nm, G, rq, rkb, psb in (
                        ("c", G1, rcol["qc"], rrow["kt"], pct_sb),
                        ("t", G2, rcol["qt"], rrow["kc"], ptt_sb)):
                    L = spool.tile([D, D], f32, tag=f"L{attn_nm}{b}")
                    nc.vector.tensor_scalar(out=L[:], in0=G, scalar1=rq[:],
                                            scalar2=None,
                                            op0=mybir.AluOpType.mult)
                    nc.vector.tensor_tensor(out=L[:], in0=L[:], in1=rkb[:],
                                            op=mybir.AluOpType.mult)
                    A = spool.tile([D, D], bf16, tag=f"A{attn_nm}{b}")
                    nc.vector.memset(A[:], 0.0)
                    for h in range(HEADS):
                        p0 = 32 * h
                        blk = L[p0:p0 + 32, p0:p0 + 32]
                        nmax = spool.tile([32, 1], f32, tag=f"nm{attn_nm}{b}{h}")
                        nc.vector.tensor_reduce(out=nmax[:], in_=blk,
                                                axis=mybir.AxisListType.X,
                                                op=mybir.AluOpType.max,
                                                negate=True)
                        e = spool.tile([32, 32], f32, tag=f"e{attn_nm}{b}{h}")
                        nc.scalar.activation(e[:], blk,
                                             mybir.ActivationFunctionType.Exp,
                                             bias=nmax[:], scale=1.0)
                        ssum = spool.tile([32, 1], f32, tag=f"ss{attn_nm}{b}{h}")
                        nc.vector.tensor_reduce(out=ssum[:], in_=e[:],
                                                axis=mybir.AxisListType.X,
                                                op=mybir.AluOpType.add)
                        rs = spool.tile([32, 1], f32, tag=f"rs{attn_nm}{b}{h}")
                        nc.vector.reciprocal(rs[:], ssum[:])
                        nc.vector.tensor_scalar(out=A[p0:p0 + 32, p0:p0 + 32],
                                                in0=e[:], scalar1=rs[:],
                                                scalar2=None,
                                                op0=mybir.AluOpType.mult)
                    # M^T = A(lhsT) . P^T  -> [d, o]
                    mps = wps.tile([D, D], f32, tag="apsum")
                    nc.tensor.matmul(mps[:], A[:], psb[:], start=True, stop=True)
                    msb = spool.tile([D, D], bf16, tag=f"m{attn_nm}{b}")
                    nc.vector.tensor_copy(msb[:], mps[:])
                    mt[(b, attn_nm)] = msb

            # ---- final: out = M_cT^T @ v_t + M_tT^T @ v_c + bias ----
            # Stage per-batch output in SBUF (bf16), track per-channel
            # absmax, then quantize to int8 with per-channel scale.
            for b in range(B):
                ob = opool.tile([D, RPC * W], bf16, tag="ob", name=f"ob{b}")
                amax = spool.tile([D, 1], f32, tag=f"amax{b}")
                for ck in range(NCHK):
                    ps = wps.tile([D, CHK], f32, tag="apsum")
                    sl = slice(ck * CHK, (ck + 1) * CHK)
                    nc.tensor.matmul(ps[:], mt[(b, "c")][:], v_sb[(b, "lo")][:, sl],
                                     start=True, stop=False)
                    nc.tensor.matmul(ps[:], mt[(b, "t")][:], v_sb[(b, "hi")][:, sl],
                                     start=False, stop=True)
                    nc.scalar.activation(ob[:, sl], ps[:],
                                         mybir.ActivationFunctionType.Identity,
                                         bias=biasv_sb[:], scale=1.0)
                    oabs = spool.tile([D, CHK], f32, tag="oabs")
                    nc.scalar.activation(oabs[:], ps[:],
                                         mybir.ActivationFunctionType.Abs,
                                         bias=biasv_sb[:], scale=1.0)
                    cmax = spool.tile([D, 1], f32, tag=f"cmax{b}")
                    nc.vector.tensor_reduce(out=cmax[:], in_=oabs[:],
                                            axis=mybir.AxisListType.X,
                                            op=mybir.AluOpType.max)
                    if ck == 0:
                        nc.vector.tensor_copy(amax[:], cmax[:])
                    else:
                        nc.vector.tensor_tensor(out=amax[:], in0=amax[:],
                                                in1=cmax[:],
                                                op=mybir.AluOpType.max)
                # scale = amax/127 (host dequant), rscale = 127/amax
                scl = spool.tile([D, 1], f32, tag=f"scl{b}")
                nc.vector.tensor_scalar(out=scl[:], in0=amax[:],
                                        scalar1=1.0 / 127.0, scalar2=None,
                                        op0=mybir.AluOpType.mult)
                nc.sync.dma_start(out=oq8[b][:, RPC * W:OCOL],
                                  in_=scl[:].bitcast(i8))
                rsc = spool.tile([D, 1], f32, tag=f"rsc{b}")
                nc.vector.reciprocal(rsc[:], scl[:])
                oq = opool.tile([D, RPC * W], i8, tag="oq", name=f"oq{b}")
                nc.scalar.activation(oq[:], ob[:],
                                     mybir.ActivationFunctionType.Identity,
                                     bias=0.0, scale=rsc[:])
                nc.sync.dma_start(out=oq8[b][:, 0:RPC * W], in_=oq[:])

    nc.compile()
    return nc


def _get_nc():
    if "nc" not in _CACHE:
        from concourse import bass, bacc, tile, mybir
        _CACHE["mods"] = (bass, bacc, tile, mybir)
        _CACHE["nc"] = _build(_CACHE["mods"])
    return _CACHE["nc"]


def _quant_in(x):
    """x: [B,D,H,W] f32 -> (int8 row-padded shards per core, scales [B,D] f32)."""
    x = np.asarray(x, np.float32)
    sc = np.abs(x).max(axis=(2, 3), keepdims=True) / 127.0  # [B,D,1,1]
    xq = np.clip(np.round(x / sc), -127, 127).astype(np.int8)
    xp = np.zeros((B, D, H + 2, W), np.int8)
    xp[:, :, 1:H + 1, :] = xq
    sh = []
    for c in range(NC):
        r0 = c * RPC
        sh.append(np.ascontiguousarray(xp[:, :, r0:r0 + HR, :]))
    return sh, np.ascontiguousarray(sc[:, :, 0, 0])


def _prep_inputs(low, high, temperature, qc_w, qdw_c_w, kvc_w, kvdw_c_w,
                 qt_w, qdw_t_w, kvt_w, kvdw_t_w, po_c_w, po_t_w,
                 concat_w, concat_b):
    """Host-side weight folding + input shard/pad/quant. Returns in_maps."""
    W3 = {
        "q_hi": _fold3x3(qc_w, qdw_c_w),
        "k_hi": _fold3x3(kvc_w[:96], kvdw_c_w[:96]),
        "v_hi": _fold3x3(kvc_w[96:], kvdw_c_w[96:]),
        "q_lo": _fold3x3(qt_w, qdw_t_w),
        "k_lo": _fold3x3(kvt_w[:96], kvdw_t_w[:96]),
        "v_lo": _fold3x3(kvt_w[96:], kvdw_t_w[96:]),
    }
    wqk_hi = _bf16(np.concatenate([W3["q_hi"], W3["k_hi"]], axis=2))  # [9,96,192]
    wqk_lo = _bf16(np.concatenate([W3["k_lo"], W3["q_lo"]], axis=2))
    wv_hi = _bf16(W3["v_hi"])
    wv_lo = _bf16(W3["v_lo"])
    # device layout [D(ci), 9, O]
    wqk_hi = np.ascontiguousarray(wqk_hi.transpose(1, 0, 2))
    wqk_lo = np.ascontiguousarray(wqk_lo.transpose(1, 0, 2))
    wv_hi = np.ascontiguousarray(wv_hi.transpose(1, 0, 2))
    wv_lo = np.ascontiguousarray(wv_lo.transpose(1, 0, 2))
    P_c = concat_w[:, :96] @ po_c_w
    P_t = concat_w[:, 96:] @ po_t_w
    pct = _bf16(P_c.T)
    ptt = _bf16(P_t.T)
    ident = _bf16(np.eye(D, dtype=np.float32))
    tempv = np.repeat(np.asarray(temperature, np.float32).reshape(3), 32)[:, None]
    biasv = np.asarray(concat_b, np.float32)[:, None]

    # pack all bf16 weights into one flat buffer, split 8 ways
    wflat = np.concatenate([
        wqk_hi.ravel(), wqk_lo.ravel(), wv_hi.ravel(), wv_lo.ravel(),
        pct.ravel(), ptt.ravel(), ident.ravel()])
    assert wflat.size == WTOT
    wshards = [np.ascontiguousarray(wflat[c * WSH:(c + 1) * WSH].reshape(1, WSH))
               for c in range(NC)]

    lo_sh, lo_sc = _quant_in(low)
    hi_sh, hi_sc = _quant_in(high)
    smalls = np.concatenate([hi_sc.ravel(), lo_sc.ravel(),
                             tempv.ravel(), biasv.ravel()]).astype(np.float32)

    in_maps = []
    for c in range(NC):
        blob = np.empty((1, NB), np.int8)
        fl = blob[0]
        fl[0:2 * XSZ1] = hi_sh[c].reshape(-1).view(np.int8)
        fl[2 * XSZ1:4 * XSZ1] = lo_sh[c].reshape(-1).view(np.int8)
        fl[WOFF:WOFF + WSH * 2] = wshards[c].reshape(-1).view(np.int8)
        fl[SOFF:NB] = smalls.view(np.int8)
        in_maps.append({"xin": blob})
    return in_maps


def run(trace=False, in_maps=None, **inputs):
    import time as _time
    from concourse.bass_utils import run_bass_kernel_spmd
    nc = _get_nc()
    if in_maps is None:
        in_maps = _prep_inputs(**inputs)
    t0 = _time.time()
    res = run_bass_kernel_spmd(nc, in_maps, list(range(NC)), trace=trace)
    res.dispatch_wall_s = _time.time() - t0
    res.in_maps = in_maps
    out = np.empty((B, D, H, W), np.float32)
    for c in range(NC):
        raw = res.results[c]["oq8"]  # [B, D, OCOL] int8
        oscl = raw[:, :, RPC * W:].copy().view(np.float32)  # [B, D, 1]
        oi = raw[:, :, :RPC * W].astype(np.float32) * oscl
        out[:, :, c * RPC:(c + 1) * RPC, :] = oi.reshape(B, D, RPC, W)
    return out, res


def kernel(**inputs):
    out, _ = run(trace=False, **inputs)
    return out


# revision 17
# speedup vs baseline: 1.4504x; 1.4504x over previous
"""Trainium2 Bass kernel for dual channel-attention block (nn_Attention_85985245266248).

Strategy:
  - Shard spatially: 256 rows -> 8 cores x 32 rows, each core's input shard
    carries a 1-row halo (zero at global edges) and 1-col zero padding.
  - Axon-tunnel traffic is the wall-clock bottleneck (~38MB/s), so all
    transfers are quantized: inputs int8 with per-(batch,channel) scales
    (dequantized on device), outputs int8 with per-(core,batch,channel)
    scales (quantized on device, round-to-nearest), weights bf16 sharded
    1/8th per core and AllGather'd on device.
  - conv1x1 + depthwise3x3 folded into a full 3x3 conv (rank-1 weights),
    executed as 9 PSUM-accumulated matmuls per tile on the PE.
  - Pass A computes q,k in [px, ch] layout (input stationary, weights moving)
    so the c-x-c Gram matrices q@k^T and the L2 norms come straight off the
    PE with pixel-contraction; partial Grams are AllReduce'd across cores.
  - Pass B computes v in [ch, px] layout (weights stationary).
  - Softmax + norm scaling on DVE/ACT (tiny 96x96 tensors).
  - Output projection po/concat folded on host into P_c/P_t; final output is
    two accumulated matmuls per pixel chunk: out = M_cT^T @ v_t + M_tT^T @ v_c + b.
All heavy matmuls run in bf16 (fp32 accumulate in PSUM).
"""
import os
import sys
import numpy as np

sys.path.insert(0, "/opt/trn_rl_repo")

# Persistent XLA compilation cache: run_bass_kernel_spmd builds a fresh jit
# closure per call, which defeats jax's in-memory executable cache and would
# re-run the NEFF compile hook on every dispatch (~0.5s). The disk cache
# makes repeat dispatches hit.
import jax as _jax
_jax.config.update("jax_compilation_cache_dir", "/tmp/jax_comp_cache")
_jax.config.update("jax_persistent_cache_min_compile_time_secs", 0.0)
_jax.config.update("jax_persistent_cache_min_entry_size_bytes", 0)

B = 2
D = 96
H = 256
W = 256
HEADS = 3
NC = 8
RPC = H // NC          # rows per core = 32
HR = RPC + 2           # halo rows = 34
PW = W + 2             # padded width = 258
PXT = 128              # pass-A pixel tile (half row)
NT_A = RPC * W // PXT  # pass-A tiles per batch per tensor = 64
CHK = 512              # pass-B / final chunk = 2 rows
NCHK = RPC * W // CHK  # 16

# flat bf16 weight-gather layout: (name, elems)
WPACK = [
    ("wqk_hi", D * 9 * 2 * D),   # 165888
    ("wqk_lo", D * 9 * 2 * D),   # 165888
    ("wv_hi", D * 9 * D),        # 82944
    ("wv_lo", D * 9 * D),        # 82944
    ("pct", D * D),              # 9216
    ("ptt", D * D),              # 9216
    ("ident", D * D),            # 9216
]
WTOT = sum(n for _, n in WPACK)  # 525312
WSH = WTOT // NC                 # 65664 per core

# single-blob input layout (bytes). One sharded array per dispatch kills
# the ~70ms-per-array axon fixed cost.
XSZ1 = D * HR * W              # one (tensor, batch) block, no col pad
WOFF = 4 * XSZ1                # bf16 weight shard bytes
SOFF = WOFF + WSH * 2          # f32 smalls: sc_hi(2), sc_lo(2), tempv, biasv
NSM = 2 * B + 2
NB = SOFF + NSM * D * 4        # total blob bytes per core
OCOL = RPC * W + 4             # int8 payload + bitcast f32 scale per row

_CACHE = {}


def _fold3x3(w1, dw):
    """w1:[O,C], dw:[O,1,3,3] -> [9, C, O] rhs-layout folded weights."""
    O, C = w1.shape
    out = np.zeros((9, C, O), np.float32)
    for t in range(9):
        dy, dx = t // 3, t % 3
        out[t] = (dw[:, 0, dy, dx][:, None] * w1).T
    return out


def _bf16(a):
    import ml_dtypes
    return np.asarray(a, np.float32).astype(ml_dtypes.bfloat16)


def _build(nc_mod):
    """Build the Bass program (uses modules passed in)."""
    bass, bacc, tile, mybir = nc_mod
    f32 = mybir.dt.float32
    bf16 = mybir.dt.bfloat16
    i8 = mybir.dt.int8

    nc = bacc.Bacc("TRN2", target_bir_lowering=False, debug=False, num_devices=NC)

    # I/O: one flat int8 blob in (x shards + bf16 weight shard + f32 smalls,
    # all bitcast), one int8 blob out (payload + bitcast f32 scale per row).
    xin = nc.dram_tensor("xin", [1, NB], i8, kind="ExternalInput")
    oq8 = nc.dram_tensor("oq8", [B, D, OCOL], i8, kind="ExternalOutput")

    NG = 6  # grams per batch: G1, G2, Sqc, Skc, Sqt, Skt

    with tile.TileContext(nc) as tc:
        with (
            tc.tile_pool(name="consts", bufs=1) as cpool,
            tc.tile_pool(name="xq", bufs=1) as xqpool,
            tc.tile_pool(name="xres", bufs=1) as xpool,
            tc.tile_pool(name="vres", bufs=1) as vpool,
            tc.tile_pool(name="qk", bufs=4) as qkpool,
            tc.tile_pool(name="work_ps", bufs=3, space="PSUM") as wps,
            tc.tile_pool(name="gram_ps", bufs=1, space="PSUM") as gps,
            tc.tile_pool(name="small", bufs=1) as spool,
            tc.tile_pool(name="obuf", bufs=1) as opool,
            tc.tile_pool(name="dram", bufs=1, space="DRAM") as dpool,
        ):
            # ---- weight AllGather: 1/8th slice per core -> full flat ----
            wsh_sb = cpool.tile([D, WSH // D], bf16, tag="wsh")
            nc.sync.dma_start(out=wsh_sb[:],
                              in_=xin[0, WOFF:WOFF + WSH * 2].bitcast(bf16))
            wag_in = dpool.tile([1, WSH], bf16, tag="wagin")
            wag_out = dpool.tile([1, WTOT], bf16, tag="wagout")
            nc.gpsimd.dma_start(out=wag_in[:], in_=wsh_sb[:])
            nc.gpsimd.collective_compute(
                "AllGather",
                mybir.AluOpType.bypass,
                replica_groups=[list(range(NC))],
                ins=[wag_in.opt()],
                outs=[wag_out.opt()],
            )

            # ---- unpack gathered weights into const tiles ----
            wqk_hi_sb = cpool.tile([D, 9, 2 * D], bf16, tag="wqkh")
            wqk_lo_sb = cpool.tile([D, 9, 2 * D], bf16, tag="wqkl")
            wv_hi_sb = cpool.tile([D, 9, D], bf16, tag="wvh")
            wv_lo_sb = cpool.tile([D, 9, D], bf16, tag="wvl")
            pct_sb = cpool.tile([D, D], bf16, tag="pct")
            ptt_sb = cpool.tile([D, D], bf16, tag="ptt")
            identb_sb = cpool.tile([D, D], bf16, tag="identb")
            wtiles = {"wqk_hi": wqk_hi_sb, "wqk_lo": wqk_lo_sb,
                      "wv_hi": wv_hi_sb, "wv_lo": wv_lo_sb,
                      "pct": pct_sb, "ptt": ptt_sb, "ident": identb_sb}
            off = 0
            for nm, n in WPACK:
                nc.gpsimd.dma_start(out=wtiles[nm][:], in_=wag_out[0, off:off + n])
                off += n
            ident_sb = cpool.tile([D, D], f32, tag="ident")
            nc.vector.tensor_copy(ident_sb[:], identb_sb[:])

            def small_slice(i):
                o = SOFF + i * D * 4
                return xin[0, o:o + D * 4].bitcast(f32)

            scq_sb = {}
            for i, (s, b) in enumerate((("hi", 0), ("hi", 1),
                                        ("lo", 0), ("lo", 1))):
                t = cpool.tile([D, 1], f32, tag=f"sc{s}{b}")
                nc.sync.dma_start(out=t[:], in_=small_slice(i))
                scq_sb[(s, b)] = t
            tempv_sb = cpool.tile([D, 1], f32, tag="tempv")
            biasv_sb = cpool.tile([D, 1], f32, tag="biasv")
            nc.sync.dma_start(out=tempv_sb[:], in_=small_slice(4))
            nc.sync.dma_start(out=biasv_sb[:], in_=small_slice(5))

            # gram accumulation targets and per-batch v stores
            gram_cat = spool.tile([D, B * NG * D], f32, tag="gramcat")
            v_sb = {}   # (b, 'hi'/'lo') -> [D, RPC*W] bf16
            for b in range(B):
                for s in ("hi", "lo"):
                    v_sb[(b, s)] = vpool.tile([D, RPC * W], bf16,
                                              tag=f"v{b}{s}", name=f"v{b}{s}")

            xt = {}
            for b in range(B):
                # ---- load + dequantize this batch's input shards ----
                for si, s in enumerate(("hi", "lo")):
                    xoff = (si * B + b) * XSZ1
                    xq = xqpool.tile([D, HR, PW], i8, tag="xq")
                    nc.vector.memset(xq[:], 0.0)
                    nc.sync.dma_start(out=xq[:, :, 1:W + 1],
                                      in_=xin[0, xoff:xoff + XSZ1])
                    xd = xpool.tile([D, HR, PW], bf16, tag=f"x{s}")
                    nc.scalar.activation(xd[:], xq[:],
                                         mybir.ActivationFunctionType.Identity,
                                         bias=0.0, scale=scq_sb[(s, b)][:])
                    xt[(b, s)] = xd
                    del xq, xd

                # ---- pass A: q,k in [px, ch] + Gram/norm accumulation ----
                # paired layout sbp[:, g, :]: g=0 -> [q_c | k_t], g=1 -> [k_c | q_t]
                gA = gps.tile([D, 2 * D], f32, tag="gA", name=f"gA{b}")  # [Sqc | G1]
                gB = gps.tile([D, 2 * D], f32, tag="gB", name=f"gB{b}")  # [G2 | Sqt]
                gC = gps.tile([D, D], f32, tag="gC", name=f"gC{b}")      # Skt
                gD = gps.tile([D, D], f32, tag="gD", name=f"gD{b}")      # Skc

                def grams(sbp, first, last):
                    nc.tensor.matmul(gA[:], sbp[:, 0, 0:D], sbp[:, 0, :],
                                     start=first, stop=last)
                    nc.tensor.matmul(gB[:], sbp[:, 1, D:2 * D], sbp[:, 1, :],
                                     start=first, stop=last)
                    nc.tensor.matmul(gC[:], sbp[:, 0, D:2 * D], sbp[:, 0, D:2 * D],
                                     start=first, stop=last)
                    nc.tensor.matmul(gD[:], sbp[:, 1, 0:D], sbp[:, 1, 0:D],
                                     start=first, stop=last)

                prev = None
                for it in range(NT_A):
                    r = (it * PXT) // W          # output row 0..31
                    j = (it * PXT) % W           # 0 or 128
                    sbp = qkpool.tile([PXT, 2, 2 * D], bf16, tag="qksb")
                    for gi, (s, wsb) in enumerate((("hi", wqk_hi_sb),
                                                   ("lo", wqk_lo_sb))):
                        ps = wps.tile([PXT, 2 * D], f32, tag="apsum")
                        xs = xt[(b, s)]
                        for t in range(9):
                            dy, dx = t // 3, t % 3
                            lhsT = xs[:, r + dy, j + dx:j + dx + PXT]
                            nc.tensor.matmul(ps[:], lhsT, wsb[:, t, :],
                                             start=(t == 0), stop=(t == 8))
                        # hi [q_c|k_c] -> cols {0:96, 192:288}; lo [k_t|q_t] -> {96:192, 288:384}
                        nc.vector.tensor_copy(sbp[:, :, gi * D:(gi + 1) * D], ps[:])
                    if prev is not None:
                        grams(prev, prev_first, False)
                    prev_first = prev is None
                    prev = sbp
                grams(prev, False, True)

                for k, src in (("G1", gA[:, D:2 * D]), ("G2", gB[:, 0:D]),
                               ("Sqc", gA[:, 0:D]), ("Skc", gD[:]),
                               ("Sqt", gB[:, D:2 * D]), ("Skt", gC[:])):
                    gi = ("G1", "G2", "Sqc", "Skc", "Sqt", "Skt").index(k)
                    off = (b * NG + gi) * D
                    nc.vector.tensor_copy(gram_cat[:, off:off + D], src)

                # ---- pass B: v in [ch, px] ----
                for s, wsb in (("hi", wv_hi_sb), ("lo", wv_lo_sb)):
                    xs = xt[(b, s)]
                    for ck in range(NCHK):
                        r = ck * 2
                        ps = wps.tile([D, CHK], f32, tag="apsum")
                        for t in range(9):
                            dy, dx = t // 3, t % 3
                            rhs = xs[:, r + dy:r + dy + 2, dx:dx + W]
                            nc.tensor.matmul(ps[:], wsb[:, t, :], rhs,
                                             start=(t == 0), stop=(t == 8))
                        nc.vector.tensor_copy(
                            v_sb[(b, s)][:, ck * CHK:(ck + 1) * CHK], ps[:])

            # ---- AllReduce partial grams across the 8 cores ----
            ar_in = dpool.tile([D, B * NG * D], f32, tag="arin")
            ar_out = dpool.tile([D, B * NG * D], f32, tag="arout")
            nc.gpsimd.dma_start(out=ar_in[:], in_=gram_cat[:])
            nc.gpsimd.collective_compute(
                "AllReduce",
                mybir.AluOpType.add,
                replica_groups=[list(range(NC))],
                ins=[ar_in.opt()],
                outs=[ar_out.opt()],
            )
            gram_red = spool.tile([D, B * NG * D], f32, tag="gramred")
            nc.gpsimd.dma_start(out=gram_red[:], in_=ar_out[:])

            # ---- post-AR small compute per batch ----
            mt = {}  # (b, 'c'/'t') -> M^T tile [D, D] bf16
            for b in range(B):
                def gslice(gi):
                    off = (b * NG + gi) * D
                    return gram_red[:, off:off + D]
                G1, G2, Sqc, Skc, Sqt, Skt = [gslice(i) for i in range(NG)]

                rcol = {}
                for nm, S in (("qc", Sqc), ("kc", Skc), ("qt", Sqt), ("kt", Skt)):
                    tmp = spool.tile([D, D], f32, tag="dtmp")
                    nc.vector.tensor_tensor(out=tmp[:], in0=S, in1=ident_sb[:],
                                            op=mybir.AluOpType.mult)
                    dg = spool.tile([D, 1], f32, tag=f"d{nm}{b}")
                    nc.vector.tensor_reduce(out=dg[:], in_=tmp[:],
                                            axis=mybir.AxisListType.X,
                                            op=mybir.AluOpType.add)
                    sq = spool.tile([D, 1], f32, tag=f"sq{nm}{b}")
                    nc.scalar.sqrt(sq[:], dg[:])
                    rc = spool.tile([D, 1], f32, tag=f"rc{nm}{b}")
                    nc.vector.reciprocal(rc[:], sq[:])
                    rcol[nm] = rc
                # fold temperature into rq
                for nm in ("qc", "qt"):
                    nc.vector.tensor_tensor(out=rcol[nm][:], in0=rcol[nm][:],
                                            in1=tempv_sb[:],
                                            op=mybir.AluOpType.mult)

                # row-vector 1/||k|| via partition reduce of (S*I)
                rrow = {}
                for nm, S in (("kt", Skt), ("kc", Skc)):
                    tmp = spool.tile([D, D], f32, tag="dtmp")
                    nc.vector.tensor_tensor(out=tmp[:], in0=S, in1=ident_sb[:],
                                            op=mybir.AluOpType.mult)
                    drow = spool.tile([1, D], f32, tag=f"dr{nm}{b}")
                    nc.gpsimd.tensor_reduce(out=drow[:], in_=tmp[:],
                                            axis=mybir.AxisListType.C,
                                            op=mybir.AluOpType.add)
                    sqr = spool.tile([1, D], f32, tag=f"sqr{nm}{b}")
                    nc.scalar.sqrt(sqr[:], drow[:])
                    rr = spool.tile([1, D], f32, tag=f"rr{nm}{b}")
                    nc.vector.reciprocal(rr[:], sqr[:])
                    rb = spool.tile([D, D], f32, tag=f"rb{nm}{b}")
                    nc.gpsimd.partition_broadcast(rb[:], rr[:])
                    rrow[nm] = rb

                for attn_nm, G, rq, rkb, psb in (
                        ("c", G1, rcol["qc"], rrow["kt"], pct_sb),
                        ("t", G2, rcol["qt"], rrow["kc"], ptt_sb)):
                    L = spool.tile([D, D], f32, tag=f"L{attn_nm}{b}")
                    nc.vector.tensor_scalar(out=L[:], in0=G, scalar1=rq[:],
                                            scalar2=None,
                                            op0=mybir.AluOpType.mult)
                    nc.vector.tensor_tensor(out=L[:], in0=L[:], in1=rkb[:],
                                            op=mybir.AluOpType.mult)
                    A = spool.tile([D, D], bf16, tag=f"A{attn_nm}{b}")
                    nc.vector.memset(A[:], 0.0)
                    for h in range(HEADS):
                        p0 = 32 * h
                        blk = L[p0:p0 + 32, p0:p0 + 32]
                        nmax = spool.tile([32, 1], f32, tag=f"nm{attn_nm}{b}{h}")
                        nc.vector.tensor_reduce(out=nmax[:], in_=blk,
                                                axis=mybir.AxisListType.X,
                                                op=mybir.AluOpType.max,
                                                negate=True)
                        e = spool.tile([32, 32], f32, tag=f"e{attn_nm}{b}{h}")
                        nc.scalar.activation(e[:], blk,
                                             mybir.ActivationFunctionType.Exp,
                                             bias=nmax[:], scale=1.0)
                        ssum = spool.tile([32, 1], f32, tag=f"ss{attn_nm}{b}{h}")
                        nc.vector.tensor_reduce(out=ssum[:], in_=e[:],
                                                axis=mybir.AxisListType.X,
                                                op=mybir.AluOpType.add)
                        rs = spool.tile([32, 1], f32, tag=f"rs{attn_nm}{b}{h}")
                        nc.vector.reciprocal(rs[:], ssum[:])
                        nc.vector.tensor_scalar(out=A[p0:p0 + 32, p0:p0 + 32],
                                                in0=e[:], scalar1=rs[:],
                                                scalar2=None,
                                                op0=mybir.AluOpType.mult)
                    # M^T = A(lhsT) . P^T  -> [d, o]
                    mps = wps.tile([D, D], f32, tag="apsum")
                    nc.tensor.matmul(mps[:], A[:], psb[:], start=True, stop=True)
                    msb = spool.tile([D, D], bf16, tag=f"m{attn_nm}{b}")
                    nc.vector.tensor_copy(msb[:], mps[:])
                    mt[(b, attn_nm)] = msb

            # ---- final: out = M_cT^T @ v_t + M_tT^T @ v_c + bias ----
            # Stage per-batch output in SBUF (bf16), track per-channel
            # absmax, then quantize to int8 with per-channel scale.
            for b in range(B):
                ob = opool.tile([D, RPC * W], bf16, tag="ob", name=f"ob{b}")
                amax = spool.tile([D, 1], f32, tag=f"amax{b}")
                for ck in range(NCHK):
                    ps = wps.tile([D, CHK], f32, tag="apsum")
                    sl = slice(ck * CHK, (ck + 1) * CHK)
                    nc.tensor.matmul(ps[:], mt[(b, "c")][:], v_sb[(b, "lo")][:, sl],
                                     start=True, stop=False)
                    nc.tensor.matmul(ps[:], mt[(b, "t")][:], v_sb[(b, "hi")][:, sl],
                                     start=False, stop=True)
                    nc.scalar.activation(ob[:, sl], ps[:],
                                         mybir.ActivationFunctionType.Identity,
                                         bias=biasv_sb[:], scale=1.0)
                    oabs = spool.tile([D, CHK], f32, tag="oabs")
                    nc.scalar.activation(oabs[:], ps[:],
                                         mybir.ActivationFunctionType.Abs,
                                         bias=biasv_sb[:], scale=1.0)
                    cmax = spool.tile([D, 1], f32, tag=f"cmax{b}")
                    nc.vector.tensor_reduce(out=cmax[:], in_=oabs[:],
                                            axis=mybir.AxisListType.X,
                                            op=mybir.AluOpType.max)
                    if ck == 0:
                        nc.vector.tensor_copy(amax[:], cmax[:])
                    else:
                        nc.vector.tensor_tensor(out=amax[:], in0=amax[:],
                                                in1=cmax[:],
                                                op=mybir.AluOpType.max)
                # scale = amax/127 (host dequant), rscale = 127/amax
                scl = spool.tile([D, 1], f32, tag=f"scl{b}")
                nc.vector.tensor_scalar(out=scl[:], in0=amax[:],
                                        scalar1=1.0 / 127.0, scalar2=None,
                                        op0=mybir.AluOpType.mult)
                nc.sync.dma_start(out=oq8[b][:, RPC * W:OCOL],
                                  in_=scl[:].bitcast(i8))
                rsc = spool.tile([D, 1], f32, tag=f"rsc{b}")
                nc.vector.reciprocal(rsc[:], scl[:])
                oq = opool.tile([D, RPC * W], i8, tag="oq", name=f"oq{b}")
                nc.scalar.activation(oq[:], ob[:],
                                     mybir.ActivationFunctionType.Identity,
                                     bias=0.0, scale=rsc[:])
                nc.sync.dma_start(out=oq8[b][:, 0:RPC * W], in_=oq[:])

    nc.compile()
    return nc


def _get_nc():
    if "nc" not in _CACHE:
        from concourse import bass, bacc, tile, mybir
        _CACHE["mods"] = (bass, bacc, tile, mybir)
        _CACHE["nc"] = _build(_CACHE["mods"])
    return _CACHE["nc"]


def _quant_in(x):
    """x: [B,D,H,W] f32 -> (int8 row-padded shards per core, scales [B,D] f32)."""
    x = np.asarray(x, np.float32)
    sc = np.abs(x).max(axis=(2, 3), keepdims=True) / 127.0  # [B,D,1,1]
    xq = np.clip(np.round(x / sc), -127, 127).astype(np.int8)
    xp = np.zeros((B, D, H + 2, W), np.int8)
    xp[:, :, 1:H + 1, :] = xq
    sh = []
    for c in range(NC):
        r0 = c * RPC
        sh.append(np.ascontiguousarray(xp[:, :, r0:r0 + HR, :]))
    return sh, np.ascontiguousarray(sc[:, :, 0, 0])


def _prep_inputs(low, high, temperature, qc_w, qdw_c_w, kvc_w, kvdw_c_w,
                 qt_w, qdw_t_w, kvt_w, kvdw_t_w, po_c_w, po_t_w,
                 concat_w, concat_b):
    """Host-side weight folding + input shard/pad/quant. Returns in_maps."""
    W3 = {
        "q_hi": _fold3x3(qc_w, qdw_c_w),
        "k_hi": _fold3x3(kvc_w[:96], kvdw_c_w[:96]),
        "v_hi": _fold3x3(kvc_w[96:], kvdw_c_w[96:]),
        "q_lo": _fold3x3(qt_w, qdw_t_w),
        "k_lo": _fold3x3(kvt_w[:96], kvdw_t_w[:96]),
        "v_lo": _fold3x3(kvt_w[96:], kvdw_t_w[96:]),
    }
    wqk_hi = _bf16(np.concatenate([W3["q_hi"], W3["k_hi"]], axis=2))  # [9,96,192]
    wqk_lo = _bf16(np.concatenate([W3["k_lo"], W3["q_lo"]], axis=2))
    wv_hi = _bf16(W3["v_hi"])
    wv_lo = _bf16(W3["v_lo"])
    # device layout [D(ci), 9, O]
    wqk_hi = np.ascontiguousarray(wqk_hi.transpose(1, 0, 2))
    wqk_lo = np.ascontiguousarray(wqk_lo.transpose(1, 0, 2))
    wv_hi = np.ascontiguousarray(wv_hi.transpose(1, 0, 2))
    wv_lo = np.ascontiguousarray(wv_lo.transpose(1, 0, 2))
    P_c = concat_w[:, :96] @ po_c_w
    P_t = concat_w[:, 96:] @ po_t_w
    pct = _bf16(P_c.T)
    ptt = _bf16(P_t.T)
    ident = _bf16(np.eye(D, dtype=np.float32))
    tempv = np.repeat(np.asarray(temperature, np.float32).reshape(3), 32)[:, None]
    biasv = np.asarray(concat_b, np.float32)[:, None]

    # pack all bf16 weights into one flat buffer, split 8 ways
    wflat = np.concatenate([
        wqk_hi.ravel(), wqk_lo.ravel(), wv_hi.ravel(), wv_lo.ravel(),
        pct.ravel(), ptt.ravel(), ident.ravel()])
    assert wflat.size == WTOT
    wshards = [np.ascontiguousarray(wflat[c * WSH:(c + 1) * WSH].reshape(1, WSH))
               for c in range(NC)]

    lo_sh, lo_sc = _quant_in(low)
    hi_sh, hi_sc = _quant_in(high)
    smalls = np.concatenate([hi_sc.ravel(), lo_sc.ravel(),
                             tempv.ravel(), biasv.ravel()]).astype(np.float32)

    in_maps = []
    for c in range(NC):
        blob = np.empty((1, NB), np.int8)
        fl = blob[0]
        fl[0:2 * XSZ1] = hi_sh[c].reshape(-1).view(np.int8)
        fl[2 * XSZ1:4 * XSZ1] = lo_sh[c].reshape(-1).view(np.int8)
        fl[WOFF:WOFF + WSH * 2] = wshards[c].reshape(-1).view(np.int8)
        fl[SOFF:NB] = smalls.view(np.int8)
        in_maps.append({"xin": blob})
    return in_maps


def run(trace=False, in_maps=None, **inputs):
    import time as _time
    from concourse.bass_utils import run_bass_kernel_spmd
    nc = _get_nc()
    if in_maps is None:
        in_maps = _prep_inputs(**inputs)
    t0 = _time.time()
    res = run_bass_kernel_spmd(nc, in_maps, list(range(NC)), trace=trace)
    res.dispatch_wall_s = _time.time() - t0
    res.in_maps = in_maps
    out = np.empty((B, D, H, W), np.float32)
    for c in range(NC):
        raw = res.results[c]["oq8"]  # [B, D, OCOL] int8
        oscl = raw[:, :, RPC * W:].copy().view(np.float32)  # [B, D, 1]
        oi = raw[:, :, :RPC * W].astype(np.float32) * oscl
        out[:, :, c * RPC:(c + 1) * RPC, :] = oi.reshape(B, D, RPC, W)
    return out, res


def kernel(**inputs):
    out, _ = run(trace=False, **inputs)
    return out


# revision 22
# speedup vs baseline: 1.5655x; 1.0794x over previous
"""Trainium2 Bass kernel for dual channel-attention block (nn_Attention_85985245266248).

Strategy:
  - Shard spatially: 256 rows -> 8 cores x 32 rows, each core's input shard
    carries a 1-row halo (zero at global edges) and 1-col zero padding.
  - Axon-tunnel traffic is the wall-clock bottleneck (~38MB/s), so all
    transfers are quantized: inputs int8 with per-(batch,channel) scales
    (dequantized on device), outputs int8 with per-(core,batch,channel)
    scales (quantized on device, round-to-nearest), weights bf16 sharded
    1/8th per core and AllGather'd on device.
  - conv1x1 + depthwise3x3 folded into a full 3x3 conv (rank-1 weights),
    executed as 9 PSUM-accumulated matmuls per tile on the PE.
  - Pass A computes q,k in [px, ch] layout (input stationary, weights moving)
    so the c-x-c Gram matrices q@k^T and the L2 norms come straight off the
    PE with pixel-contraction; partial Grams are AllReduce'd across cores.
  - Pass B computes v in [ch, px] layout (weights stationary).
  - Softmax + norm scaling on DVE/ACT (tiny 96x96 tensors).
  - Output projection po/concat folded on host into P_c/P_t; final output is
    two accumulated matmuls per pixel chunk: out = M_cT^T @ v_t + M_tT^T @ v_c + b.
All heavy matmuls run in bf16 (fp32 accumulate in PSUM).
"""
import os
import sys
import numpy as np

sys.path.insert(0, "/opt/trn_rl_repo")

# Persistent XLA compilation cache: run_bass_kernel_spmd builds a fresh jit
# closure per call, which defeats jax's in-memory executable cache and would
# re-run the NEFF compile hook on every dispatch (~0.5s). The disk cache
# makes repeat dispatches hit.
import jax as _jax
_jax.config.update("jax_compilation_cache_dir", "/tmp/jax_comp_cache")
_jax.config.update("jax_persistent_cache_min_compile_time_secs", 0.0)
_jax.config.update("jax_persistent_cache_min_entry_size_bytes", 0)

B = 2
D = 96
H = 256
W = 256
HEADS = 3
NC = 8
RPC = H // NC          # rows per core = 32
HR = RPC + 2           # halo rows = 34
PW = W + 2             # padded width = 258
PXT = 128              # pass-A pixel tile (half row)
NT_A = RPC * W // PXT  # pass-A tiles per batch per tensor = 64
CHK = 512              # pass-B / final chunk = 2 rows
NCHK = RPC * W // CHK  # 16

# flat bf16 weight-gather layout: (name, elems)
WPACK = [
    ("wqk_hi", D * 9 * 2 * D),   # 165888
    ("wqk_lo", D * 9 * 2 * D),   # 165888
    ("wv_hi", D * 9 * D),        # 82944
    ("wv_lo", D * 9 * D),        # 82944
    ("pct", D * D),              # 9216
    ("ptt", D * D),              # 9216
    ("ident", D * D),            # 9216
]
WTOT = sum(n for _, n in WPACK)  # 525312
WSH = WTOT // NC                 # 65664 per core

# single-blob input layout (bytes). One sharded array per dispatch kills
# the ~70ms-per-array axon fixed cost. Halo rows are exchanged on-device
# (AllGather of boundary rows + dynamic-offset DMA), not shipped.
XSZ1 = D * RPC * W             # one (tensor, batch) block, exact rows
WOFF = 4 * XSZ1                # bf16 weight shard bytes
SOFF = WOFF + WSH * 2          # f32 smalls: sc_hi(2), sc_lo(2), tempv, biasv
NSM = 2 * B + 2
NB = SOFF + NSM * D * 4        # total blob bytes per core
OCOL = RPC * W + 4             # int8 payload + bitcast f32 scale per row
CBC = 4 * 2 * W                # contrib cols/partition: 4 blocks x {top,bot} x W

_CACHE = {}


def _fold3x3(w1, dw):
    """w1:[O,C], dw:[O,1,3,3] -> [9, C, O] rhs-layout folded weights."""
    O, C = w1.shape
    out = np.zeros((9, C, O), np.float32)
    for t in range(9):
        dy, dx = t // 3, t % 3
        out[t] = (dw[:, 0, dy, dx][:, None] * w1).T
    return out


def _bf16(a):
    import ml_dtypes
    return np.asarray(a, np.float32).astype(ml_dtypes.bfloat16)


def _build(nc_mod):
    """Build the Bass program (uses modules passed in)."""
    bass, bacc, tile, mybir = nc_mod
    f32 = mybir.dt.float32
    bf16 = mybir.dt.bfloat16
    i8 = mybir.dt.int8

    nc = bacc.Bacc("TRN2", target_bir_lowering=False, debug=False, num_devices=NC)

    # I/O: one flat int8 blob in (x shards + bf16 weight shard + f32 smalls,
    # all bitcast), one int8 blob out (payload + bitcast f32 scale per row).
    xin = nc.dram_tensor("xin", [1, NB], i8, kind="ExternalInput")
    oq8 = nc.dram_tensor("oq8", [B, D, OCOL], i8, kind="ExternalOutput")

    NG = 6  # grams per batch: G1, G2, Sqc, Skc, Sqt, Skt

    with tile.TileContext(nc) as tc:
        with (
            tc.tile_pool(name="consts", bufs=1) as cpool,
            tc.tile_pool(name="xq", bufs=1) as xqpool,
            tc.tile_pool(name="xres", bufs=1) as xpool,
            tc.tile_pool(name="vres", bufs=1) as vpool,
            tc.tile_pool(name="qk", bufs=4) as qkpool,
            tc.tile_pool(name="work_ps", bufs=3, space="PSUM") as wps,
            tc.tile_pool(name="gram_ps", bufs=1, space="PSUM") as gps,
            tc.tile_pool(name="small", bufs=1) as spool,
            tc.tile_pool(name="obuf", bufs=1) as opool,
            tc.tile_pool(name="dram", bufs=1, space="DRAM") as dpool,
        ):
            # ---- weight AllGather: 1/8th slice per core -> full flat ----
            wsh_sb = cpool.tile([D, WSH // D], bf16, tag="wsh")
            nc.sync.dma_start(out=wsh_sb[:],
                              in_=xin[0, WOFF:WOFF + WSH * 2].bitcast(bf16))
            wag_in = dpool.tile([1, WSH], bf16, tag="wagin")
            wag_out = dpool.tile([1, WTOT], bf16, tag="wagout")
            nc.gpsimd.dma_start(out=wag_in[:], in_=wsh_sb[:])
            nc.gpsimd.collective_compute(
                "AllGather",
                mybir.AluOpType.bypass,
                replica_groups=[list(range(NC))],
                ins=[wag_in.opt()],
                outs=[wag_out.opt()],
            )

            # ---- unpack gathered weights into const tiles ----
            wqk_hi_sb = cpool.tile([D, 9, 2 * D], bf16, tag="wqkh")
            wqk_lo_sb = cpool.tile([D, 9, 2 * D], bf16, tag="wqkl")
            wv_hi_sb = cpool.tile([D, 9, D], bf16, tag="wvh")
            wv_lo_sb = cpool.tile([D, 9, D], bf16, tag="wvl")
            pct_sb = cpool.tile([D, D], bf16, tag="pct")
            ptt_sb = cpool.tile([D, D], bf16, tag="ptt")
            identb_sb = cpool.tile([D, D], bf16, tag="identb")
            wtiles = {"wqk_hi": wqk_hi_sb, "wqk_lo": wqk_lo_sb,
                      "wv_hi": wv_hi_sb, "wv_lo": wv_lo_sb,
                      "pct": pct_sb, "ptt": ptt_sb, "ident": identb_sb}
            off = 0
            for nm, n in WPACK:
                nc.gpsimd.dma_start(out=wtiles[nm][:], in_=wag_out[0, off:off + n])
                off += n
            ident_sb = cpool.tile([D, D], f32, tag="ident")
            nc.vector.tensor_copy(ident_sb[:], identb_sb[:])

            def small_slice(i):
                o = SOFF + i * D * 4
                return xin[0, o:o + D * 4].bitcast(f32)

            scq_sb = {}
            for i, (s, b) in enumerate((("hi", 0), ("hi", 1),
                                        ("lo", 0), ("lo", 1))):
                t = cpool.tile([D, 1], f32, tag=f"sc{s}{b}")
                nc.sync.dma_start(out=t[:], in_=small_slice(i))
                scq_sb[(s, b)] = t
            tempv_sb = cpool.tile([D, 1], f32, tag="tempv")
            biasv_sb = cpool.tile([D, 1], f32, tag="biasv")
            nc.sync.dma_start(out=tempv_sb[:], in_=small_slice(4))
            nc.sync.dma_start(out=biasv_sb[:], in_=small_slice(5))

            # ---- halo exchange: AllGather boundary rows, dynamic-offset pick ----
            # Each core contributes its top+bottom rows per (tensor, batch)
            # block; core c then fetches core c-1's bottom / c+1's top row via
            # runtime-offset DMA. Global edges go OOB and skip, leaving zeros.
            cb = spool.tile([D, CBC], i8, tag="cb")
            for si in range(2):
                for b in range(B):
                    idx = si * B + b
                    blk = xin[0, idx * XSZ1:(idx + 1) * XSZ1].rearrange(
                        "(c r w) -> c r w", c=D, r=RPC, w=W)
                    c0 = idx * 2 * W
                    nc.sync.dma_start(out=cb[:, c0:c0 + W], in_=blk[:, 0, :])
                    nc.sync.dma_start(out=cb[:, c0 + W:c0 + 2 * W],
                                      in_=blk[:, RPC - 1, :])
            hg_in = dpool.tile([D, CBC], i8, tag="hgin")
            hg_out = dpool.tile([NC * D, CBC], i8, tag="hgout")
            nc.gpsimd.dma_start(out=hg_in[:], in_=cb[:])
            nc.gpsimd.collective_compute(
                "AllGather",
                mybir.AluOpType.bypass,
                replica_groups=[list(range(NC))],
                ins=[hg_in.opt()],
                outs=[hg_out.opt()],
            )
            # pad with zeroed guard blocks so pid 0/7 reads hit zeros (the
            # reference zero-pads at global edges) and offsets stay in range
            hg_pad = dpool.tile([(NC + 2) * D, CBC], i8, tag="hgpad")
            zrow = spool.tile([D, CBC], i8, tag="zrow")
            nc.vector.memset(zrow[:], 0.0)
            nc.sync.dma_start(out=hg_pad[0:D], in_=zrow[:])
            nc.sync.dma_start(out=hg_pad[(NC + 1) * D:(NC + 2) * D], in_=zrow[:])
            nc.gpsimd.dma_start(out=hg_pad[D:(NC + 1) * D], in_=hg_out[:])
            hstage = spool.tile([D, 4, 2, W], i8, tag="hstage")
            pid = nc.sync.partition_id()
            for idx in range(4):
                c0 = idx * 2 * W
                nc.sync.dma_start(
                    out=hstage[:, idx, 0, :],
                    in_=hg_pad[bass.ds(pid * D, D), c0 + W:c0 + 2 * W])
                nc.sync.dma_start(
                    out=hstage[:, idx, 1, :],
                    in_=hg_pad[bass.ds((pid + 2) * D, D), c0:c0 + W])

            # gram accumulation targets and per-batch v stores
            gram_cat = spool.tile([D, B * NG * D], f32, tag="gramcat")
            v_sb = {}   # (b, 'hi'/'lo') -> [D, RPC*W] bf16
            for b in range(B):
                for s in ("hi", "lo"):
                    v_sb[(b, s)] = vpool.tile([D, RPC * W], bf16,
                                              tag=f"v{b}{s}", name=f"v{b}{s}")

            xt = {}
            for b in range(B):
                # ---- load + dequantize this batch's input shards ----
                for si, s in enumerate(("hi", "lo")):
                    idx = si * B + b
                    xoff = idx * XSZ1
                    xq = xqpool.tile([D, HR, PW], i8, tag="xq")
                    nc.vector.memset(xq[:], 0.0)
                    nc.sync.dma_start(out=xq[:, 1:RPC + 1, 1:W + 1],
                                      in_=xin[0, xoff:xoff + XSZ1])
                    xd = xpool.tile([D, HR, PW], bf16, tag=f"x{s}")
                    nc.scalar.activation(xd[:], xq[:],
                                         mybir.ActivationFunctionType.Identity,
                                         bias=0.0, scale=scq_sb[(s, b)][:])
                    # overwrite halo rows with dequantized neighbor rows
                    nc.vector.tensor_scalar(
                        out=xd[:, 0, 1:W + 1], in0=hstage[:, idx, 0, :],
                        scalar1=scq_sb[(s, b)][:], scalar2=None,
                        op0=mybir.AluOpType.mult)
                    nc.vector.tensor_scalar(
                        out=xd[:, HR - 1, 1:W + 1], in0=hstage[:, idx, 1, :],
                        scalar1=scq_sb[(s, b)][:], scalar2=None,
                        op0=mybir.AluOpType.mult)
                    xt[(b, s)] = xd
                    del xq, xd

                # ---- pass A: q,k in [px, ch] + Gram/norm accumulation ----
                # paired layout sbp[:, g, :]: g=0 -> [q_c | k_t], g=1 -> [k_c | q_t]
                gA = gps.tile([D, 2 * D], f32, tag="gA", name=f"gA{b}")  # [Sqc | G1]
                gB = gps.tile([D, 2 * D], f32, tag="gB", name=f"gB{b}")  # [G2 | Sqt]
                gC = gps.tile([D, D], f32, tag="gC", name=f"gC{b}")      # Skt
                gD = gps.tile([D, D], f32, tag="gD", name=f"gD{b}")      # Skc

                def grams(sbp, first, last):
                    nc.tensor.matmul(gA[:], sbp[:, 0, 0:D], sbp[:, 0, :],
                                     start=first, stop=last)
                    nc.tensor.matmul(gB[:], sbp[:, 1, D:2 * D], sbp[:, 1, :],
                                     start=first, stop=last)
                    nc.tensor.matmul(gC[:], sbp[:, 0, D:2 * D], sbp[:, 0, D:2 * D],
                                     start=first, stop=last)
                    nc.tensor.matmul(gD[:], sbp[:, 1, 0:D], sbp[:, 1, 0:D],
                                     start=first, stop=last)

                prev = None
                for it in range(NT_A):
                    r = (it * PXT) // W          # output row 0..31
                    j = (it * PXT) % W           # 0 or 128
                    sbp = qkpool.tile([PXT, 2, 2 * D], bf16, tag="qksb")
                    for gi, (s, wsb) in enumerate((("hi", wqk_hi_sb),
                                                   ("lo", wqk_lo_sb))):
                        ps = wps.tile([PXT, 2 * D], f32, tag="apsum")
                        xs = xt[(b, s)]
                        for t in range(9):
                            dy, dx = t // 3, t % 3
                            lhsT = xs[:, r + dy, j + dx:j + dx + PXT]
                            nc.tensor.matmul(ps[:], lhsT, wsb[:, t, :],
                                             start=(t == 0), stop=(t == 8))
                        # hi [q_c|k_c] -> cols {0:96, 192:288}; lo [k_t|q_t] -> {96:192, 288:384}
                        nc.vector.tensor_copy(sbp[:, :, gi * D:(gi + 1) * D], ps[:])
                    if prev is not None:
                        grams(prev, prev_first, False)
                    prev_first = prev is None
                    prev = sbp
                grams(prev, False, True)

                for k, src in (("G1", gA[:, D:2 * D]), ("G2", gB[:, 0:D]),
                               ("Sqc", gA[:, 0:D]), ("Skc", gD[:]),
                               ("Sqt", gB[:, D:2 * D]), ("Skt", gC[:])):
                    gi = ("G1", "G2", "Sqc", "Skc", "Sqt", "Skt").index(k)
                    off = (b * NG + gi) * D
                    nc.vector.tensor_copy(gram_cat[:, off:off + D], src)

                # ---- pass B: v in [ch, px] ----
                for s, wsb in (("hi", wv_hi_sb), ("lo", wv_lo_sb)):
                    xs = xt[(b, s)]
                    for ck in range(NCHK):
                        r = ck * 2
                        ps = wps.tile([D, CHK], f32, tag="apsum")
                        for t in range(9):
                            dy, dx = t // 3, t % 3
                            rhs = xs[:, r + dy:r + dy + 2, dx:dx + W]
                            nc.tensor.matmul(ps[:], wsb[:, t, :], rhs,
                                             start=(t == 0), stop=(t == 8))
                        nc.vector.tensor_copy(
                            v_sb[(b, s)][:, ck * CHK:(ck + 1) * CHK], ps[:])

            # ---- AllReduce partial grams across the 8 cores ----
            ar_in = dpool.tile([D, B * NG * D], f32, tag="arin")
            ar_out = dpool.tile([D, B * NG * D], f32, tag="arout")
            nc.gpsimd.dma_start(out=ar_in[:], in_=gram_cat[:])
            nc.gpsimd.collective_compute(
                "AllReduce",
                mybir.AluOpType.add,
                replica_groups=[list(range(NC))],
                ins=[ar_in.opt()],
                outs=[ar_out.opt()],
            )
            gram_red = spool.tile([D, B * NG * D], f32, tag="gramred")
            nc.gpsimd.dma_start(out=gram_red[:], in_=ar_out[:])

            # ---- post-AR small compute per batch ----
            mt = {}  # (b, 'c'/'t') -> M^T tile [D, D] bf16
            for b in range(B):
                def gslice(gi):
                    off = (b * NG + gi) * D
                    return gram_red[:, off:off + D]
                G1, G2, Sqc, Skc, Sqt, Skt = [gslice(i) for i in range(NG)]

                rcol = {}
                for nm, S in (("qc", Sqc), ("kc", Skc), ("qt", Sqt), ("kt", Skt)):
                    tmp = spool.tile([D, D], f32, tag="dtmp")
                    nc.vector.tensor_tensor(out=tmp[:], in0=S, in1=ident_sb[:],
                                            op=mybir.AluOpType.mult)
                    dg = spool.tile([D, 1], f32, tag=f"d{nm}{b}")
                    nc.vector.tensor_reduce(out=dg[:], in_=tmp[:],
                                            axis=mybir.AxisListType.X,
                                            op=mybir.AluOpType.add)
                    sq = spool.tile([D, 1], f32, tag=f"sq{nm}{b}")
                    nc.scalar.sqrt(sq[:], dg[:])
                    rc = spool.tile([D, 1], f32, tag=f"rc{nm}{b}")
                    nc.vector.reciprocal(rc[:], sq[:])
                    rcol[nm] = rc
                # fold temperature into rq
                for nm in ("qc", "qt"):
                    nc.vector.tensor_tensor(out=rcol[nm][:], in0=rcol[nm][:],
                                            in1=tempv_sb[:],
                                            op=mybir.AluOpType.mult)

                # row-vector 1/||k|| via partition reduce of (S*I)
                rrow = {}
                for nm, S in (("kt", Skt), ("kc", Skc)):
                    tmp = spool.tile([D, D], f32, tag="dtmp")
                    nc.vector.tensor_tensor(out=tmp[:], in0=S, in1=ident_sb[:],
                                            op=mybir.AluOpType.mult)
                    drow = spool.tile([1, D], f32, tag=f"dr{nm}{b}")
                    nc.gpsimd.tensor_reduce(out=drow[:], in_=tmp[:],
                                            axis=mybir.AxisListType.C,
                                            op=mybir.AluOpType.add)
                    sqr = spool.tile([1, D], f32, tag=f"sqr{nm}{b}")
                    nc.scalar.sqrt(sqr[:], drow[:])
                    rr = spool.tile([1, D], f32, tag=f"rr{nm}{b}")
                    nc.vector.reciprocal(rr[:], sqr[:])
                    rb = spool.tile([D, D], f32, tag=f"rb{nm}{b}")
                    nc.gpsimd.partition_broadcast(rb[:], rr[:])
                    rrow[nm] = rb

                for attn_nm, G, rq, rkb, psb in (
                        ("c", G1, rcol["qc"], rrow["kt"], pct_sb),
                        ("t", G2, rcol["qt"], rrow["kc"], ptt_sb)):
                    L = spool.tile([D, D], f32, tag=f"L{attn_nm}{b}")
                    nc.vector.tensor_scalar(out=L[:], in0=G, scalar1=rq[:],
                                            scalar2=None,
                                            op0=mybir.AluOpType.mult)
                    nc.vector.tensor_tensor(out=L[:], in0=L[:], in1=rkb[:],
                                            op=mybir.AluOpType.mult)
                    A = spool.tile([D, D], bf16, tag=f"A{attn_nm}{b}")
                    nc.vector.memset(A[:], 0.0)
                    for h in range(HEADS):
                        p0 = 32 * h
                        blk = L[p0:p0 + 32, p0:p0 + 32]
                        nmax = spool.tile([32, 1], f32, tag=f"nm{attn_nm}{b}{h}")
                        nc.vector.tensor_reduce(out=nmax[:], in_=blk,
                                                axis=mybir.AxisListType.X,
                                                op=mybir.AluOpType.max,
                                                negate=True)
                        e = spool.tile([32, 32], f32, tag=f"e{attn_nm}{b}{h}")
                        nc.scalar.activation(e[:], blk,
                                             mybir.ActivationFunctionType.Exp,
                                             bias=nmax[:], scale=1.0)
                        ssum = spool.tile([32, 1], f32, tag=f"ss{attn_nm}{b}{h}")
                        nc.vector.tensor_reduce(out=ssum[:], in_=e[:],
                                                axis=mybir.AxisListType.X,
                                                op=mybir.AluOpType.add)
                        rs = spool.tile([32, 1], f32, tag=f"rs{attn_nm}{b}{h}")
                        nc.vector.reciprocal(rs[:], ssum[:])
                        nc.vector.tensor_scalar(out=A[p0:p0 + 32, p0:p0 + 32],
                                                in0=e[:], scalar1=rs[:],
                                                scalar2=None,
                                                op0=mybir.AluOpType.mult)
                    # M^T = A(lhsT) . P^T  -> [d, o]
                    mps = wps.tile([D, D], f32, tag="apsum")
                    nc.tensor.matmul(mps[:], A[:], psb[:], start=True, stop=True)
                    msb = spool.tile([D, D], bf16, tag=f"m{attn_nm}{b}")
                    nc.vector.tensor_copy(msb[:], mps[:])
                    mt[(b, attn_nm)] = msb

            # ---- final: out = M_cT^T @ v_t + M_tT^T @ v_c + bias ----
            # Stage per-batch output in SBUF (bf16), track per-channel
            # absmax, then quantize to int8 with per-channel scale.
            for b in range(B):
                ob = opool.tile([D, RPC * W], bf16, tag="ob", name=f"ob{b}")
                amax = spool.tile([D, 1], f32, tag=f"amax{b}")
                for ck in range(NCHK):
                    ps = wps.tile([D, CHK], f32, tag="apsum")
                    sl = slice(ck * CHK, (ck + 1) * CHK)
                    nc.tensor.matmul(ps[:], mt[(b, "c")][:], v_sb[(b, "lo")][:, sl],
                                     start=True, stop=False)
                    nc.tensor.matmul(ps[:], mt[(b, "t")][:], v_sb[(b, "hi")][:, sl],
                                     start=False, stop=True)
                    nc.scalar.activation(ob[:, sl], ps[:],
                                         mybir.ActivationFunctionType.Identity,
                                         bias=biasv_sb[:], scale=1.0)
                    oabs = spool.tile([D, CHK], f32, tag="oabs")
                    nc.scalar.activation(oabs[:], ps[:],
                                         mybir.ActivationFunctionType.Abs,
                                         bias=biasv_sb[:], scale=1.0)
                    cmax = spool.tile([D, 1], f32, tag=f"cmax{b}")
                    nc.vector.tensor_reduce(out=cmax[:], in_=oabs[:],
                                            axis=mybir.AxisListType.X,
                                            op=mybir.AluOpType.max)
                    if ck == 0:
                        nc.vector.tensor_copy(amax[:], cmax[:])
                    else:
                        nc.vector.tensor_tensor(out=amax[:], in0=amax[:],
                                                in1=cmax[:],
                                                op=mybir.AluOpType.max)
                # scale = amax/127 (host dequant), rscale = 127/amax
                scl = spool.tile([D, 1], f32, tag=f"scl{b}")
                nc.vector.tensor_scalar(out=scl[:], in0=amax[:],
                                        scalar1=1.0 / 127.0, scalar2=None,
                                        op0=mybir.AluOpType.mult)
                nc.sync.dma_start(out=oq8[b][:, RPC * W:OCOL],
                                  in_=scl[:].bitcast(i8))
                rsc = spool.tile([D, 1], f32, tag=f"rsc{b}")
                nc.vector.reciprocal(rsc[:], scl[:])
                oq = opool.tile([D, RPC * W], i8, tag="oq", name=f"oq{b}")
                nc.scalar.activation(oq[:], ob[:],
                                     mybir.ActivationFunctionType.Identity,
                                     bias=0.0, scale=rsc[:])
                nc.sync.dma_start(out=oq8[b][:, 0:RPC * W], in_=oq[:])

    nc.compile()
    return nc


def _get_nc():
    if "nc" not in _CACHE:
        from concourse import bass, bacc, tile, mybir
        _CACHE["mods"] = (bass, bacc, tile, mybir)
        _CACHE["nc"] = _build(_CACHE["mods"])
    return _CACHE["nc"]


def _quant_in(x):
    """x: [B,D,H,W] f32 -> (int8 exact shards per core, scales [B,D] f32)."""
    x = np.asarray(x, np.float32)
    sc = np.abs(x).max(axis=(2, 3), keepdims=True) / 127.0  # [B,D,1,1]
    xq = np.clip(np.round(x / sc), -127, 127).astype(np.int8)
    sh = [np.ascontiguousarray(xq[:, :, c * RPC:(c + 1) * RPC, :])
          for c in range(NC)]
    return sh, np.ascontiguousarray(sc[:, :, 0, 0])


def _prep_inputs(low, high, temperature, qc_w, qdw_c_w, kvc_w, kvdw_c_w,
                 qt_w, qdw_t_w, kvt_w, kvdw_t_w, po_c_w, po_t_w,
                 concat_w, concat_b):
    """Host-side weight folding + input shard/pad/quant. Returns in_maps."""
    W3 = {
        "q_hi": _fold3x3(qc_w, qdw_c_w),
        "k_hi": _fold3x3(kvc_w[:96], kvdw_c_w[:96]),
        "v_hi": _fold3x3(kvc_w[96:], kvdw_c_w[96:]),
        "q_lo": _fold3x3(qt_w, qdw_t_w),
        "k_lo": _fold3x3(kvt_w[:96], kvdw_t_w[:96]),
        "v_lo": _fold3x3(kvt_w[96:], kvdw_t_w[96:]),
    }
    wqk_hi = _bf16(np.concatenate([W3["q_hi"], W3["k_hi"]], axis=2))  # [9,96,192]
    wqk_lo = _bf16(np.concatenate([W3["k_lo"], W3["q_lo"]], axis=2))
    wv_hi = _bf16(W3["v_hi"])
    wv_lo = _bf16(W3["v_lo"])
    # device layout [D(ci), 9, O]
    wqk_hi = np.ascontiguousarray(wqk_hi.transpose(1, 0, 2))
    wqk_lo = np.ascontiguousarray(wqk_lo.transpose(1, 0, 2))
    wv_hi = np.ascontiguousarray(wv_hi.transpose(1, 0, 2))
    wv_lo = np.ascontiguousarray(wv_lo.transpose(1, 0, 2))
    P_c = concat_w[:, :96] @ po_c_w
    P_t = concat_w[:, 96:] @ po_t_w
    pct = _bf16(P_c.T)
    ptt = _bf16(P_t.T)
    ident = _bf16(np.eye(D, dtype=np.float32))
    tempv = np.repeat(np.asarray(temperature, np.float32).reshape(3), 32)[:, None]
    biasv = np.asarray(concat_b, np.float32)[:, None]

    # pack all bf16 weights into one flat buffer, split 8 ways
    wflat = np.concatenate([
        wqk_hi.ravel(), wqk_lo.ravel(), wv_hi.ravel(), wv_lo.ravel(),
        pct.ravel(), ptt.ravel(), ident.ravel()])
    assert wflat.size == WTOT
    wshards = [np.ascontiguousarray(wflat[c * WSH:(c + 1) * WSH].reshape(1, WSH))
               for c in range(NC)]

    lo_sh, lo_sc = _quant_in(low)
    hi_sh, hi_sc = _quant_in(high)
    smalls = np.concatenate([hi_sc.ravel(), lo_sc.ravel(),
                             tempv.ravel(), biasv.ravel()]).astype(np.float32)

    in_maps = []
    for c in range(NC):
        blob = np.empty((1, NB), np.int8)
        fl = blob[0]
        fl[0:2 * XSZ1] = hi_sh[c].reshape(-1).view(np.int8)
        fl[2 * XSZ1:4 * XSZ1] = lo_sh[c].reshape(-1).view(np.int8)
        fl[WOFF:WOFF + WSH * 2] = wshards[c].reshape(-1).view(np.int8)
        fl[SOFF:NB] = smalls.view(np.int8)
        in_maps.append({"xin": blob})
    return in_maps


def run(trace=False, in_maps=None, **inputs):
    import time as _time
    from concourse.bass_utils import run_bass_kernel_spmd
    nc = _get_nc()
    if in_maps is None:
        in_maps = _prep_inputs(**inputs)
    t0 = _time.time()
    res = run_bass_kernel_spmd(nc, in_maps, list(range(NC)), trace=trace)
    res.dispatch_wall_s = _time.time() - t0
    res.in_maps = in_maps
    out = np.empty((B, D, H, W), np.float32)
    for c in range(NC):
        raw = res.results[c]["oq8"]  # [B, D, OCOL] int8
        oscl = raw[:, :, RPC * W:].copy().view(np.float32)  # [B, D, 1]
        oi = raw[:, :, :RPC * W].astype(np.float32) * oscl
        out[:, :, c * RPC:(c + 1) * RPC, :] = oi.reshape(B, D, RPC, W)
    return out, res


def kernel(**inputs):
    out, _ = run(trace=False, **inputs)
    return out


# revision 23
# speedup vs baseline: 1.6130x; 1.0303x over previous
"""Trainium2 Bass kernel for dual channel-attention block (nn_Attention_85985245266248).

Strategy:
  - Shard spatially: 256 rows -> 8 cores x 32 rows, each core's input shard
    carries a 1-row halo (zero at global edges) and 1-col zero padding.
  - Axon-tunnel traffic is the wall-clock bottleneck (~38MB/s), so all
    transfers are quantized: inputs int8 with per-(batch,channel) scales
    (dequantized on device), outputs int8 with per-(core,batch,channel)
    scales (quantized on device, round-to-nearest), weights bf16 sharded
    1/8th per core and AllGather'd on device.
  - conv1x1 + depthwise3x3 folded into a full 3x3 conv (rank-1 weights),
    executed as 9 PSUM-accumulated matmuls per tile on the PE.
  - Pass A computes q,k in [px, ch] layout (input stationary, weights moving)
    so the c-x-c Gram matrices q@k^T and the L2 norms come straight off the
    PE with pixel-contraction; partial Grams are AllReduce'd across cores.
  - Pass B computes v in [ch, px] layout (weights stationary).
  - Softmax + norm scaling on DVE/ACT (tiny 96x96 tensors).
  - Output projection po/concat folded on host into P_c/P_t; final output is
    two accumulated matmuls per pixel chunk: out = M_cT^T @ v_t + M_tT^T @ v_c + b.
All heavy matmuls run in bf16 (fp32 accumulate in PSUM).
"""
import os
import sys
import numpy as np

sys.path.insert(0, "/opt/trn_rl_repo")

# Persistent XLA compilation cache: run_bass_kernel_spmd builds a fresh jit
# closure per call, which defeats jax's in-memory executable cache and would
# re-run the NEFF compile hook on every dispatch (~0.5s). The disk cache
# makes repeat dispatches hit.
import jax as _jax
_jax.config.update("jax_compilation_cache_dir", "/tmp/jax_comp_cache")
_jax.config.update("jax_persistent_cache_min_compile_time_secs", 0.0)
_jax.config.update("jax_persistent_cache_min_entry_size_bytes", 0)

B = 2
D = 96
H = 256
W = 256
HEADS = 3
NC = 8
RPC = H // NC          # rows per core = 32
HR = RPC + 2           # halo rows = 34
PW = W + 2             # padded width = 258
PXT = 128              # pass-A pixel tile (half row)
NT_A = RPC * W // PXT  # pass-A tiles per batch per tensor = 64
CHK = 512              # pass-B / final chunk = 2 rows
NCHK = RPC * W // CHK  # 16

# flat bf16 weight-gather layout: (name, elems)
WPACK = [
    ("wqk_hi", D * 9 * 2 * D),   # 165888
    ("wqk_lo", D * 9 * 2 * D),   # 165888
    ("wv_hi", D * 9 * D),        # 82944
    ("wv_lo", D * 9 * D),        # 82944
    ("pct", D * D),              # 9216
    ("ptt", D * D),              # 9216
    ("ident", D * D),            # 9216
]
WTOT = sum(n for _, n in WPACK)  # 525312
WSH = WTOT // NC                 # 65664 per core

# single-blob input layout (bytes). One sharded array per dispatch kills
# the ~70ms-per-array axon fixed cost. Halo rows are exchanged on-device
# (AllGather of boundary rows + dynamic-offset DMA), not shipped.
XSZ1 = D * RPC * W             # one (tensor, batch) block, exact rows
WOFF = 4 * XSZ1                # bf16 weight shard bytes
SOFF = WOFF + WSH * 2          # f32 smalls: sc_hi(2), sc_lo(2), tempv, biasv
NSM = 2 * B + 2
NB = SOFF + NSM * D * 4        # total blob bytes per core
OCOL = RPC * W + 4             # int8 payload + bitcast f32 scale per row
CBC = 4 * 2 * W                # contrib cols/partition: 4 blocks x {top,bot} x W

_CACHE = {}


def _fold3x3(w1, dw):
    """w1:[O,C], dw:[O,1,3,3] -> [9, C, O] rhs-layout folded weights."""
    O, C = w1.shape
    out = np.zeros((9, C, O), np.float32)
    for t in range(9):
        dy, dx = t // 3, t % 3
        out[t] = (dw[:, 0, dy, dx][:, None] * w1).T
    return out


def _bf16(a):
    import ml_dtypes
    return np.asarray(a, np.float32).astype(ml_dtypes.bfloat16)


def _build(nc_mod):
    """Build the Bass program (uses modules passed in)."""
    bass, bacc, tile, mybir = nc_mod
    f32 = mybir.dt.float32
    bf16 = mybir.dt.bfloat16
    i8 = mybir.dt.int8

    nc = bacc.Bacc("TRN2", target_bir_lowering=False, debug=False, num_devices=NC)

    # I/O: one flat int8 blob in (x shards + bf16 weight shard + f32 smalls,
    # all bitcast), one int8 blob out (payload + bitcast f32 scale per row).
    xin = nc.dram_tensor("xin", [1, NB], i8, kind="ExternalInput")
    oq8 = nc.dram_tensor("oq8", [B, D, OCOL], i8, kind="ExternalOutput")

    NG = 6  # grams per batch: G1, G2, Sqc, Skc, Sqt, Skt

    with tile.TileContext(nc) as tc:
        with (
            tc.tile_pool(name="consts", bufs=1) as cpool,
            tc.tile_pool(name="xq", bufs=1) as xqpool,
            tc.tile_pool(name="xres", bufs=1) as xpool,
            tc.tile_pool(name="vres", bufs=1) as vpool,
            tc.tile_pool(name="qk", bufs=4) as qkpool,
            tc.tile_pool(name="work_ps", bufs=3, space="PSUM") as wps,
            tc.tile_pool(name="gram_ps", bufs=1, space="PSUM") as gps,
            tc.tile_pool(name="small", bufs=1) as spool,
            tc.tile_pool(name="obuf", bufs=1) as opool,
            tc.tile_pool(name="dram", bufs=1, space="DRAM") as dpool,
        ):
            # ---- weight AllGather: 1/8th slice per core -> full flat ----
            wsh_sb = cpool.tile([D, WSH // D], bf16, tag="wsh")
            nc.sync.dma_start(out=wsh_sb[:],
                              in_=xin[0, WOFF:WOFF + WSH * 2].bitcast(bf16))
            wag_in = dpool.tile([1, WSH], bf16, tag="wagin")
            wag_out = dpool.tile([1, WTOT], bf16, tag="wagout")
            nc.gpsimd.dma_start(out=wag_in[:], in_=wsh_sb[:])
            nc.gpsimd.collective_compute(
                "AllGather",
                mybir.AluOpType.bypass,
                replica_groups=[list(range(NC))],
                ins=[wag_in.opt()],
                outs=[wag_out.opt()],
            )

            # ---- unpack gathered weights into const tiles ----
            wqk_hi_sb = cpool.tile([D, 9, 2 * D], bf16, tag="wqkh")
            wqk_lo_sb = cpool.tile([D, 9, 2 * D], bf16, tag="wqkl")
            wv_hi_sb = cpool.tile([D, 9, D], bf16, tag="wvh")
            wv_lo_sb = cpool.tile([D, 9, D], bf16, tag="wvl")
            pct_sb = cpool.tile([D, D], bf16, tag="pct")
            ptt_sb = cpool.tile([D, D], bf16, tag="ptt")
            identb_sb = cpool.tile([D, D], bf16, tag="identb")
            wtiles = {"wqk_hi": wqk_hi_sb, "wqk_lo": wqk_lo_sb,
                      "wv_hi": wv_hi_sb, "wv_lo": wv_lo_sb,
                      "pct": pct_sb, "ptt": ptt_sb, "ident": identb_sb}
            off = 0
            for nm, n in WPACK:
                nc.gpsimd.dma_start(out=wtiles[nm][:], in_=wag_out[0, off:off + n])
                off += n
            ident_sb = cpool.tile([D, D], f32, tag="ident")
            nc.vector.tensor_copy(ident_sb[:], identb_sb[:])

            def small_slice(i):
                o = SOFF + i * D * 4
                return xin[0, o:o + D * 4].bitcast(f32)

            scq_sb = {}
            for i, (s, b) in enumerate((("hi", 0), ("hi", 1),
                                        ("lo", 0), ("lo", 1))):
                t = cpool.tile([D, 1], f32, tag=f"sc{s}{b}")
                nc.sync.dma_start(out=t[:], in_=small_slice(i))
                scq_sb[(s, b)] = t
            tempv_sb = cpool.tile([D, 1], f32, tag="tempv")
            biasv_sb = cpool.tile([D, 1], f32, tag="biasv")
            nc.sync.dma_start(out=tempv_sb[:], in_=small_slice(4))
            nc.sync.dma_start(out=biasv_sb[:], in_=small_slice(5))

            # ---- halo exchange: AllGather boundary rows, dynamic-offset pick ----
            # Each core contributes its top+bottom rows per (tensor, batch)
            # block; core c then fetches core c-1's bottom / c+1's top row via
            # runtime-offset DMA. Global edges go OOB and skip, leaving zeros.
            cb = spool.tile([D, CBC], i8, tag="cb")
            for si in range(2):
                for b in range(B):
                    idx = si * B + b
                    blk = xin[0, idx * XSZ1:(idx + 1) * XSZ1].rearrange(
                        "(c r w) -> c r w", c=D, r=RPC, w=W)
                    c0 = idx * 2 * W
                    nc.sync.dma_start(out=cb[:, c0:c0 + W], in_=blk[:, 0, :])
                    nc.sync.dma_start(out=cb[:, c0 + W:c0 + 2 * W],
                                      in_=blk[:, RPC - 1, :])
            hg_in = dpool.tile([D, CBC], i8, tag="hgin")
            hg_out = dpool.tile([NC * D, CBC], i8, tag="hgout")
            nc.gpsimd.dma_start(out=hg_in[:], in_=cb[:])
            nc.gpsimd.collective_compute(
                "AllGather",
                mybir.AluOpType.bypass,
                replica_groups=[list(range(NC))],
                ins=[hg_in.opt()],
                outs=[hg_out.opt()],
            )
            # pad with zeroed guard blocks so pid 0/7 reads hit zeros (the
            # reference zero-pads at global edges) and offsets stay in range
            hg_pad = dpool.tile([(NC + 2) * D, CBC], i8, tag="hgpad")
            zrow = spool.tile([D, CBC], i8, tag="zrow")
            nc.vector.memset(zrow[:], 0.0)
            nc.sync.dma_start(out=hg_pad[0:D], in_=zrow[:])
            nc.sync.dma_start(out=hg_pad[(NC + 1) * D:(NC + 2) * D], in_=zrow[:])
            nc.gpsimd.dma_start(out=hg_pad[D:(NC + 1) * D], in_=hg_out[:])
            hstage = spool.tile([D, 4, 2, W], i8, tag="hstage")
            pid = nc.sync.partition_id()
            for idx in range(4):
                c0 = idx * 2 * W
                nc.sync.dma_start(
                    out=hstage[:, idx, 0, :],
                    in_=hg_pad[bass.ds(pid * D, D), c0 + W:c0 + 2 * W])
                nc.sync.dma_start(
                    out=hstage[:, idx, 1, :],
                    in_=hg_pad[bass.ds((pid + 2) * D, D), c0:c0 + W])

            # gram accumulation targets and per-batch v stores
            gram_cat = spool.tile([D, B * NG * D], f32, tag="gramcat")
            v_sb = {}   # (b, 'hi'/'lo') -> [D, RPC*W] bf16
            for b in range(B):
                for s in ("hi", "lo"):
                    v_sb[(b, s)] = vpool.tile([D, RPC * W], bf16,
                                              tag=f"v{b}{s}", name=f"v{b}{s}")

            xt = {}
            for b in range(B):
                # ---- load + dequantize this batch's input shards ----
                for si, s in enumerate(("hi", "lo")):
                    idx = si * B + b
                    xoff = idx * XSZ1
                    xq = xqpool.tile([D, HR, PW], i8, tag="xq")
                    nc.vector.memset(xq[:], 0.0)
                    nc.sync.dma_start(out=xq[:, 1:RPC + 1, 1:W + 1],
                                      in_=xin[0, xoff:xoff + XSZ1])
                    xd = xpool.tile([D, HR, PW], bf16, tag=f"x{s}")
                    nc.scalar.activation(xd[:], xq[:],
                                         mybir.ActivationFunctionType.Identity,
                                         bias=0.0, scale=scq_sb[(s, b)][:])
                    # overwrite halo rows with dequantized neighbor rows
                    nc.vector.tensor_scalar(
                        out=xd[:, 0, 1:W + 1], in0=hstage[:, idx, 0, :],
                        scalar1=scq_sb[(s, b)][:], scalar2=None,
                        op0=mybir.AluOpType.mult)
                    nc.vector.tensor_scalar(
                        out=xd[:, HR - 1, 1:W + 1], in0=hstage[:, idx, 1, :],
                        scalar1=scq_sb[(s, b)][:], scalar2=None,
                        op0=mybir.AluOpType.mult)
                    xt[(b, s)] = xd
                    del xq, xd

                # ---- pass A: q,k in [px, ch] + Gram/norm accumulation ----
                # paired layout sbp[:, g, :]: g=0 -> [q_c | k_t], g=1 -> [k_c | q_t]
                gA = gps.tile([D, 2 * D], f32, tag="gA", name=f"gA{b}")  # [Sqc | G1]
                gB = gps.tile([D, 2 * D], f32, tag="gB", name=f"gB{b}")  # [G2 | Sqt]
                gC = gps.tile([D, D], f32, tag="gC", name=f"gC{b}")      # Skt
                gD = gps.tile([D, D], f32, tag="gD", name=f"gD{b}")      # Skc

                def grams(sbp, first, last):
                    nc.tensor.matmul(gA[:], sbp[:, 0, 0:D], sbp[:, 0, :],
                                     start=first, stop=last)
                    nc.tensor.matmul(gB[:], sbp[:, 1, D:2 * D], sbp[:, 1, :],
                                     start=first, stop=last)
                    nc.tensor.matmul(gC[:], sbp[:, 0, D:2 * D], sbp[:, 0, D:2 * D],
                                     start=first, stop=last)
                    nc.tensor.matmul(gD[:], sbp[:, 1, 0:D], sbp[:, 1, 0:D],
                                     start=first, stop=last)

                prev = None
                for it in range(NT_A):
                    r = (it * PXT) // W          # output row 0..31
                    j = (it * PXT) % W           # 0 or 128
                    sbp = qkpool.tile([PXT, 2, 2 * D], bf16, tag="qksb")
                    for gi, (s, wsb) in enumerate((("hi", wqk_hi_sb),
                                                   ("lo", wqk_lo_sb))):
                        ps = wps.tile([PXT, 2 * D], f32, tag="apsum")
                        xs = xt[(b, s)]
                        for t in range(9):
                            dy, dx = t // 3, t % 3
                            lhsT = xs[:, r + dy, j + dx:j + dx + PXT]
                            nc.tensor.matmul(ps[:], lhsT, wsb[:, t, :],
                                             start=(t == 0), stop=(t == 8))
                        # hi [q_c|k_c] -> cols {0:96, 192:288}; lo [k_t|q_t] -> {96:192, 288:384}
                        nc.vector.tensor_copy(sbp[:, :, gi * D:(gi + 1) * D], ps[:])
                    if prev is not None:
                        grams(prev, prev_first, False)
                    prev_first = prev is None
                    prev = sbp
                grams(prev, False, True)

                for k, src in (("G1", gA[:, D:2 * D]), ("G2", gB[:, 0:D]),
                               ("Sqc", gA[:, 0:D]), ("Skc", gD[:]),
                               ("Sqt", gB[:, D:2 * D]), ("Skt", gC[:])):
                    gi = ("G1", "G2", "Sqc", "Skc", "Sqt", "Skt").index(k)
                    off = (b * NG + gi) * D
                    nc.vector.tensor_copy(gram_cat[:, off:off + D], src)

                # ---- pass B: v in [ch, px] ----
                for s, wsb in (("hi", wv_hi_sb), ("lo", wv_lo_sb)):
                    xs = xt[(b, s)]
                    for ck in range(NCHK):
                        r = ck * 2
                        ps = wps.tile([D, CHK], f32, tag="apsum")
                        for t in range(9):
                            dy, dx = t // 3, t % 3
                            rhs = xs[:, r + dy:r + dy + 2, dx:dx + W]
                            nc.tensor.matmul(ps[:], wsb[:, t, :], rhs,
                                             start=(t == 0), stop=(t == 8))
                        nc.vector.tensor_copy(
                            v_sb[(b, s)][:, ck * CHK:(ck + 1) * CHK], ps[:])

            # ---- AllReduce partial grams across the 8 cores ----
            ar_in = dpool.tile([D, B * NG * D], f32, tag="arin")
            ar_out = dpool.tile([D, B * NG * D], f32, tag="arout")
            nc.gpsimd.dma_start(out=ar_in[:], in_=gram_cat[:])
            nc.gpsimd.collective_compute(
                "AllReduce",
                mybir.AluOpType.add,
                replica_groups=[list(range(NC))],
                ins=[ar_in.opt()],
                outs=[ar_out.opt()],
            )
            gram_red = spool.tile([D, B * NG * D], f32, tag="gramred")
            nc.gpsimd.dma_start(out=gram_red[:], in_=ar_out[:])

            # ---- post-AR small compute per batch ----
            mt = {}  # (b, 'c'/'t') -> M^T tile [D, D] bf16
            for b in range(B):
                def gslice(gi):
                    off = (b * NG + gi) * D
                    return gram_red[:, off:off + D]
                G1, G2, Sqc, Skc, Sqt, Skt = [gslice(i) for i in range(NG)]

                rcol = {}
                for nm, S in (("qc", Sqc), ("kc", Skc), ("qt", Sqt), ("kt", Skt)):
                    tmp = spool.tile([D, D], f32, tag="dtmp")
                    nc.vector.tensor_tensor(out=tmp[:], in0=S, in1=ident_sb[:],
                                            op=mybir.AluOpType.mult)
                    dg = spool.tile([D, 1], f32, tag=f"d{nm}{b}")
                    nc.vector.tensor_reduce(out=dg[:], in_=tmp[:],
                                            axis=mybir.AxisListType.X,
                                            op=mybir.AluOpType.add)
                    sq = spool.tile([D, 1], f32, tag=f"sq{nm}{b}")
                    nc.scalar.sqrt(sq[:], dg[:])
                    rc = spool.tile([D, 1], f32, tag=f"rc{nm}{b}")
                    nc.vector.reciprocal(rc[:], sq[:])
                    rcol[nm] = rc
                # fold temperature into rq
                for nm in ("qc", "qt"):
                    nc.vector.tensor_tensor(out=rcol[nm][:], in0=rcol[nm][:],
                                            in1=tempv_sb[:],
                                            op=mybir.AluOpType.mult)

                # row-vector 1/||k|| via partition reduce of (S*I)
                rrow = {}
                for nm, S in (("kt", Skt), ("kc", Skc)):
                    tmp = spool.tile([D, D], f32, tag="dtmp")
                    nc.vector.tensor_tensor(out=tmp[:], in0=S, in1=ident_sb[:],
                                            op=mybir.AluOpType.mult)
                    drow = spool.tile([1, D], f32, tag=f"dr{nm}{b}")
                    nc.gpsimd.tensor_reduce(out=drow[:], in_=tmp[:],
                                            axis=mybir.AxisListType.C,
                                            op=mybir.AluOpType.add)
                    sqr = spool.tile([1, D], f32, tag=f"sqr{nm}{b}")
                    nc.scalar.sqrt(sqr[:], drow[:])
                    rr = spool.tile([1, D], f32, tag=f"rr{nm}{b}")
                    nc.vector.reciprocal(rr[:], sqr[:])
                    rb = spool.tile([D, D], f32, tag=f"rb{nm}{b}")
                    nc.gpsimd.partition_broadcast(rb[:], rr[:])
                    rrow[nm] = rb

                for attn_nm, G, rq, rkb, psb in (
                        ("c", G1, rcol["qc"], rrow["kt"], pct_sb),
                        ("t", G2, rcol["qt"], rrow["kc"], ptt_sb)):
                    L = spool.tile([D, D], f32, tag=f"L{attn_nm}{b}")
                    nc.vector.tensor_scalar(out=L[:], in0=G, scalar1=rq[:],
                                            scalar2=None,
                                            op0=mybir.AluOpType.mult)
                    nc.vector.tensor_tensor(out=L[:], in0=L[:], in1=rkb[:],
                                            op=mybir.AluOpType.mult)
                    A = spool.tile([D, D], bf16, tag=f"A{attn_nm}{b}")
                    nc.vector.memset(A[:], 0.0)
                    for h in range(HEADS):
                        p0 = 32 * h
                        blk = L[p0:p0 + 32, p0:p0 + 32]
                        nmax = spool.tile([32, 1], f32, tag=f"nm{attn_nm}{b}{h}")
                        nc.vector.tensor_reduce(out=nmax[:], in_=blk,
                                                axis=mybir.AxisListType.X,
                                                op=mybir.AluOpType.max,
                                                negate=True)
                        e = spool.tile([32, 32], f32, tag=f"e{attn_nm}{b}{h}")
                        nc.scalar.activation(e[:], blk,
                                             mybir.ActivationFunctionType.Exp,
                                             bias=nmax[:], scale=1.0)
                        ssum = spool.tile([32, 1], f32, tag=f"ss{attn_nm}{b}{h}")
                        nc.vector.tensor_reduce(out=ssum[:], in_=e[:],
                                                axis=mybir.AxisListType.X,
                                                op=mybir.AluOpType.add)
                        rs = spool.tile([32, 1], f32, tag=f"rs{attn_nm}{b}{h}")
                        nc.vector.reciprocal(rs[:], ssum[:])
                        nc.vector.tensor_scalar(out=A[p0:p0 + 32, p0:p0 + 32],
                                                in0=e[:], scalar1=rs[:],
                                                scalar2=None,
                                                op0=mybir.AluOpType.mult)
                    # M^T = A(lhsT) . P^T  -> [d, o]
                    mps = wps.tile([D, D], f32, tag="apsum")
                    nc.tensor.matmul(mps[:], A[:], psb[:], start=True, stop=True)
                    msb = spool.tile([D, D], bf16, tag=f"m{attn_nm}{b}")
                    nc.vector.tensor_copy(msb[:], mps[:])
                    mt[(b, attn_nm)] = msb

            # ---- final: out = M_cT^T @ v_t + M_tT^T @ v_c + bias ----
            # Stage per-batch output in SBUF (bf16), track per-channel
            # absmax, then quantize to int8 with per-channel scale.
            for b in range(B):
                ob = opool.tile([D, RPC * W], bf16, tag="ob", name=f"ob{b}")
                amax = spool.tile([D, 1], f32, tag=f"amax{b}")
                for ck in range(NCHK):
                    ps = wps.tile([D, CHK], f32, tag="apsum")
                    sl = slice(ck * CHK, (ck + 1) * CHK)
                    nc.tensor.matmul(ps[:], mt[(b, "c")][:], v_sb[(b, "lo")][:, sl],
                                     start=True, stop=False)
                    nc.tensor.matmul(ps[:], mt[(b, "t")][:], v_sb[(b, "hi")][:, sl],
                                     start=False, stop=True)
                    nc.scalar.activation(ob[:, sl], ps[:],
                                         mybir.ActivationFunctionType.Identity,
                                         bias=biasv_sb[:], scale=1.0)
                    oabs = spool.tile([D, CHK], f32, tag="oabs")
                    nc.scalar.activation(oabs[:], ps[:],
                                         mybir.ActivationFunctionType.Abs,
                                         bias=biasv_sb[:], scale=1.0)
                    cmax = spool.tile([D, 1], f32, tag=f"cmax{b}")
                    nc.vector.tensor_reduce(out=cmax[:], in_=oabs[:],
                                            axis=mybir.AxisListType.X,
                                            op=mybir.AluOpType.max)
                    if ck == 0:
                        nc.vector.tensor_copy(amax[:], cmax[:])
                    else:
                        nc.vector.tensor_tensor(out=amax[:], in0=amax[:],
                                                in1=cmax[:],
                                                op=mybir.AluOpType.max)
                # scale = amax/127 (host dequant), rscale = 127/amax
                scl = spool.tile([D, 1], f32, tag=f"scl{b}")
                nc.vector.tensor_scalar(out=scl[:], in0=amax[:],
                                        scalar1=1.0 / 127.0, scalar2=None,
                                        op0=mybir.AluOpType.mult)
                nc.sync.dma_start(out=oq8[b][:, RPC * W:OCOL],
                                  in_=scl[:].bitcast(i8))
                rsc = spool.tile([D, 1], f32, tag=f"rsc{b}")
                nc.vector.reciprocal(rsc[:], scl[:])
                oq = opool.tile([D, RPC * W], i8, tag="oq", name=f"oq{b}")
                nc.scalar.activation(oq[:], ob[:],
                                     mybir.ActivationFunctionType.Identity,
                                     bias=0.0, scale=rsc[:])
                nc.sync.dma_start(out=oq8[b][:, 0:RPC * W], in_=oq[:])

    nc.compile()
    return nc


def _get_nc():
    if "nc" not in _CACHE:
        from concourse import bass, bacc, tile, mybir
        _CACHE["mods"] = (bass, bacc, tile, mybir)
        nc = _build(_CACHE["mods"])
        # memoize the (pure, deterministic) BIR serialization: the bass2jax
        # lowering re-serializes it on every dispatch (~40ms for 4.7MB)
        jb = nc.to_json_bytes()
        nc.to_json_bytes = lambda: jb
        _CACHE["nc"] = nc
    return _CACHE["nc"]


def _quant_in(x):
    """x: [B,D,H,W] f32 -> (int8 exact shards per core, scales [B,D] f32)."""
    x = np.asarray(x, np.float32)
    sc = np.abs(x).max(axis=(2, 3), keepdims=True) / 127.0  # [B,D,1,1]
    xq = np.clip(np.round(x / sc), -127, 127).astype(np.int8)
    sh = [np.ascontiguousarray(xq[:, :, c * RPC:(c + 1) * RPC, :])
          for c in range(NC)]
    return sh, np.ascontiguousarray(sc[:, :, 0, 0])


def _prep_inputs(low, high, temperature, qc_w, qdw_c_w, kvc_w, kvdw_c_w,
                 qt_w, qdw_t_w, kvt_w, kvdw_t_w, po_c_w, po_t_w,
                 concat_w, concat_b):
    """Host-side weight folding + input shard/pad/quant. Returns in_maps."""
    W3 = {
        "q_hi": _fold3x3(qc_w, qdw_c_w),
        "k_hi": _fold3x3(kvc_w[:96], kvdw_c_w[:96]),
        "v_hi": _fold3x3(kvc_w[96:], kvdw_c_w[96:]),
        "q_lo": _fold3x3(qt_w, qdw_t_w),
        "k_lo": _fold3x3(kvt_w[:96], kvdw_t_w[:96]),
        "v_lo": _fold3x3(kvt_w[96:], kvdw_t_w[96:]),
    }
    wqk_hi = _bf16(np.concatenate([W3["q_hi"], W3["k_hi"]], axis=2))  # [9,96,192]
    wqk_lo = _bf16(np.concatenate([W3["k_lo"], W3["q_lo"]], axis=2))
    wv_hi = _bf16(W3["v_hi"])
    wv_lo = _bf16(W3["v_lo"])
    # device layout [D(ci), 9, O]
    wqk_hi = np.ascontiguousarray(wqk_hi.transpose(1, 0, 2))
    wqk_lo = np.ascontiguousarray(wqk_lo.transpose(1, 0, 2))
    wv_hi = np.ascontiguousarray(wv_hi.transpose(1, 0, 2))
    wv_lo = np.ascontiguousarray(wv_lo.transpose(1, 0, 2))
    P_c = concat_w[:, :96] @ po_c_w
    P_t = concat_w[:, 96:] @ po_t_w
    pct = _bf16(P_c.T)
    ptt = _bf16(P_t.T)
    ident = _bf16(np.eye(D, dtype=np.float32))
    tempv = np.repeat(np.asarray(temperature, np.float32).reshape(3), 32)[:, None]
    biasv = np.asarray(concat_b, np.float32)[:, None]

    # pack all bf16 weights into one flat buffer, split 8 ways
    wflat = np.concatenate([
        wqk_hi.ravel(), wqk_lo.ravel(), wv_hi.ravel(), wv_lo.ravel(),
        pct.ravel(), ptt.ravel(), ident.ravel()])
    assert wflat.size == WTOT
    wshards = [np.ascontiguousarray(wflat[c * WSH:(c + 1) * WSH].reshape(1, WSH))
               for c in range(NC)]

    lo_sh, lo_sc = _quant_in(low)
    hi_sh, hi_sc = _quant_in(high)
    smalls = np.concatenate([hi_sc.ravel(), lo_sc.ravel(),
                             tempv.ravel(), biasv.ravel()]).astype(np.float32)

    in_maps = []
    for c in range(NC):
        blob = np.empty((1, NB), np.int8)
        fl = blob[0]
        fl[0:2 * XSZ1] = hi_sh[c].reshape(-1).view(np.int8)
        fl[2 * XSZ1:4 * XSZ1] = lo_sh[c].reshape(-1).view(np.int8)
        fl[WOFF:WOFF + WSH * 2] = wshards[c].reshape(-1).view(np.int8)
        fl[SOFF:NB] = smalls.view(np.int8)
        in_maps.append({"xin": blob})
    return in_maps


def run(trace=False, in_maps=None, **inputs):
    import time as _time
    from concourse.bass_utils import run_bass_kernel_spmd
    nc = _get_nc()
    if in_maps is None:
        in_maps = _prep_inputs(**inputs)
    t0 = _time.time()
    res = run_bass_kernel_spmd(nc, in_maps, list(range(NC)), trace=trace)
    res.dispatch_wall_s = _time.time() - t0
    res.in_maps = in_maps
    out = np.empty((B, D, H, W), np.float32)
    for c in range(NC):
        raw = res.results[c]["oq8"]  # [B, D, OCOL] int8
        oscl = raw[:, :, RPC * W:].copy().view(np.float32)  # [B, D, 1]
        oi = raw[:, :, :RPC * W].astype(np.float32) * oscl
        out[:, :, c * RPC:(c + 1) * RPC, :] = oi.reshape(B, D, RPC, W)
    return out, res


def kernel(**inputs):
    out, _ = run(trace=False, **inputs)
    return out
